# revision 38
# baseline (speedup 1.0000x reference)
"""Trainium2 Bass kernel for a single causal attention head.

Problem: x:(8,2048,1024) f32, per-head projections wq/wk/wv:(64,1024),
biases (64,). Output: softmax(causal(q k^T / sqrt(64))) @ v : (8,2048,64).

Strategy:
  - Data-parallel: batch b -> core b (8 cores, 1 batch each).
  - Host prep: x[b] transposed to xT:(1024,2048) fp16 (contraction dim D on
    SBUF partitions); Q/K weights shipped stacked as [wq|wk] (fp16, 1/sqrt(64)
    folded into wq).
  - Device (per core):
      * Input DMA split across both HWDGE rings: x quarter-chunks on the sync
        ring, weights/biases on the scalar ring, so xq0 lands ~5us earlier
        than a single serialized ring.
      * qk1 = [wq|wk]^T.T @ xT: rows 0-63 = Q^T, rows 64-127 = K^T (PSUM
        accumulate over 8 d-tiles, fp16 matmuls, N=512 chunks).
      * qk2 = half-swapped copy of qk1: K^T on rows 0-63, Q^T on rows 64-127.
        Both orders exist on both partition halves -> scores for TWO k-tiles
        run concurrently via PE row packing.
      * vT transposed back to (T,64) tiles via PE transpose, augmented with a
        ones column (softmax denominator rides along the PV matmul).
      * S^T[j,i] = sum_h K^T[h,j] Q^T[h,i] transposed-scores layout; P^T =
        exp(S^T) on ACT; causal mask = GPSIMD affine_select zeroing P^T in
        the 128x128 diagonal triangle blocks only (fully-masked column
        prefixes of diagonal tiles are skipped by S/exp/PV instead of
        computed-then-zeroed).
      * O^T_aug[65, T] accumulated in PSUM over k-tiles; row 64 = sum_j P^T.
        PSUM accumulate is per-element (has_written), so later k-tiles may
        touch columns the first matmul didn't.
      * causal skip: k-tiles entirely above the diagonal never computed;
        diagonal tiles compute only their live column suffix.
      * emission order interleaves chunk ci+1 projections ahead of chunk ci
        attention so the scalar engine (exp = the metronome) never starves.
  - Host post: out[b] = (O^T[0:64] / O^T[64:65]).T  (softmax normalization).
"""

import numpy as np

B, T, D, HD = 8, 2048, 1024, 64
P = 128          # SBUF partitions
CH = 512         # q-chunk (matmul moving dim)
NCH = T // CH    # 4
DT = D // P      # 8 d-tiles
NKT = T // P     # 16 k-tiles

LAST_RESULTS = None      # BassKernelResults of the most recent run (for test.py)


def _build_module(legalize=True):
    import concourse.bass as bass
    import concourse.mybir as mybir
    from concourse.tile import TileContext

    from concourse.masks import make_identity
    F32 = mybir.dt.float32
    F16 = mybir.dt.float16

    nc = bass.Bass("TRN2", target_bir_lowering=True)

    # All inputs host-pre-arranged to the exact SBUF layout so every DMA is
    # contiguous per partition (128 big descriptors, not 1024 tiny gathers —
    # the rearrange-in-DMA variant was descriptor-bound and took ~10us).
    xT = nc.dram_tensor("xT", (P, NCH * DT * CH), F16, kind="ExternalInput")
    # one weight blob = [w1 | wv | b1 | bv] per partition -> a single DMA
    # (each separate transfer costs ~2us completion receipt on its ring)
    WQOF = DT * P              # w1 cols
    WVOF = WQOF + DT * HD      # wv cols start at WQOF
    BOF = WVOF                 # b1 at BOF, bv at BOF+1
    wb = nc.dram_tensor("wb", (P, BOF + 2), F16, kind="ExternalInput")
    outT = nc.dram_tensor("outT", (HD + 1, T), F32, kind="ExternalOutput")

    with TileContext(nc) as tc:
        with (
            tc.tile_pool(name="const", bufs=1) as const,
            tc.tile_pool(name="acts", bufs=1) as acts,
            tc.tile_pool(name="proj_ps", bufs=1, space="PSUM") as proj_ps,
            tc.tile_pool(name="tr_ps", bufs=1, space="PSUM") as tr_ps,
            tc.tile_pool(name="s_ps", bufs=2, space="PSUM") as s_ps,
            tc.tile_pool(name="o_ps", bufs=2, space="PSUM") as o_ps,
            tc.tile_pool(name="pwork", bufs=20) as pwork,
            tc.tile_pool(name="owork", bufs=2) as owork,
        ):
            ident = const.tile([P, P], F32, name="ident")
            make_identity(nc, ident)

            # ---- weights/biases on the GPSIMD (SWDGE) ring so they drain in
            # parallel with the x quarters on the sync HWDGE ring. (The
            # scalar HWDGE ring is NOT used for input: a DIRECT2D occupies
            # the ACT queue for the whole transfer and would stall exps.)
            # Input DMA runs at the shared-HBM roofline (~310 GB/s/core with
            # all 8 cores loading at once) - 4.5MB takes ~14us no matter how
            # it's split. The blob rides the scalar ring with xq1 behind it
            # (that queue must drain before the first real exp ~17us); the
            # other chunks serialize on the sync ring in consumption order.
            wb_sb = const.tile([P, BOF + 2], F16, name="wb_sb")
            nc.scalar.dma_start(out=wb_sb[:], in_=wb[:, :])
            bias = const.tile([P, 2], F32, name="bias")
            nc.vector.tensor_copy(bias[:], wb_sb[:, BOF:BOF + 2])
            b1_sb = bias[:, 0:1]
            bv_sb = bias[:, 1:2]

            xq = []
            for ci in range(NCH):
                t = const.tile([P, DT * CH], F16, name=f"xq{ci}")
                eng = nc.scalar if ci == 1 else nc.sync
                eng.dma_start(
                    out=t[:], in_=xT[:, ci * DT * CH:(ci + 1) * DT * CH])
                xq.append(t)

            wscr = const.tile([P, 2 * P], F16, name="wscr")
            nc.vector.memset(wscr[:], 0.0)
            # dummy activation up front: walrus inserts the ~1.3us
            # ACT_TABLE_LOAD before the first exp; issuing it at t~0 hides
            # the load under the input DMA instead of the first real exp.
            edum = const.tile([P, 1], F16, name="edum")
            nc.scalar.activation(edum[:], wscr[:, 0:1],
                                 mybir.ActivationFunctionType.Exp)
            # HAM warm-up: short throwaway matmuls keep the PE active through
            # its 3.4us activity window so real matmuls run at the full
            # 2.4 GHz clock. From the bufs=2 pool so they pipeline instead of
            # serializing on slot release; sized to end about when xq0 lands.
            for wu in range(12):
                pswu = s_ps.tile([P, 2 * P], F32, name="warm", tag="sT")
                nc.tensor.matmul(pswu[:], wscr[:, 0:P], wscr[:],
                                 start=True, stop=True)

            # ---- activations ----
            qk1 = acts.tile([P, T], F16, name="qk1")
            qk2 = acts.tile([P, T], F16, name="qk2")
            # vT2: V^T for a chunk PAIR stacked on partition halves (rows
            # 0-63 = even chunk, 64-127 = odd chunk) -> one 128x128 PE
            # transpose yields TWO v_aug k-tiles.
            vT2 = acts.tile([P, T // 2], F32, name="vT2")
            v_aug = acts.tile([P, NKT, HD + 1], F16, name="v_aug")
            nc.vector.memset(v_aug[:, :, HD], 1.0)

            def qk_chunk(ci):
                cs = slice(ci * CH, (ci + 1) * CH)
                rhs = xq[ci]
                ps = proj_ps.tile([P, CH], F32, name="proj", tag="proj")
                for d in range(DT):
                    nc.tensor.matmul(ps[:], wb_sb[:, d * P:(d + 1) * P],
                                     rhs[:, d * CH:(d + 1) * CH],
                                     start=(d == 0), stop=(d == DT - 1))
                nc.vector.tensor_scalar_add(qk1[:, cs], ps[:], b1_sb)
                # half-swapped copy: qk2 = [K^T; Q^T]. 64-partition DVE ops
                # read any aligned src half and write either dest half.
                nc.vector.tensor_copy(qk2[0:HD, cs], qk1[HD:P, cs])
                nc.vector.tensor_copy(qk2[HD:P, cs], qk1[0:HD, cs])

            def v_pair(ca, cb):
                # V projections for two chunks col-packed: chunk ca on array
                # columns 0-63, chunk cb on columns 64-127 -> the matmul pairs
                # overlap in the PE array; outputs land in disjoint halves of
                # one PSUM bank. The stacked vT2 halves then transpose two
                # k-tiles per 128x128 PE transpose.
                off = (ca // 2) * CH
                psv = proj_ps.tile([P, CH], F32, name="projv", tag="proj")
                for d in range(DT):
                    wvd = wb_sb[:, WQOF + d * HD:WQOF + (d + 1) * HD]
                    nc.tensor.matmul(psv[0:HD, :], wvd,
                                     xq[ca][:, d * CH:(d + 1) * CH],
                                     start=(d == 0), stop=(d == DT - 1))
                    nc.tensor.matmul(psv[HD:P, :], wvd,
                                     xq[cb][:, d * CH:(d + 1) * CH],
                                     start=(d == 0), stop=(d == DT - 1))
                nc.vector.tensor_scalar_add(
                    vT2[0:HD, off:off + CH], psv[0:HD, :], bv_sb[0:HD])
                nc.vector.tensor_scalar_add(
                    vT2[HD:P, off:off + CH], psv[HD:P, :], bv_sb[HD:P])
                for i in range(4):
                    tp = tr_ps.tile([P, 2, HD], F32, name="vtr", tag="vtr")
                    nc.tensor.transpose(tp[:, :, :],
                                        vT2[:, off + i * P:off + (i + 1) * P],
                                        ident[:, :])
                    nc.vector.tensor_copy(
                        v_aug[:, 4 * ca + i:4 * cb + i + 1:4, 0:HD], tp[:, :, :])

            def _tri_mask(pt, col):
                # zero the strictly-above-diagonal half of a 128x128 block
                # whose query offset equals its key offset (iota = c - j).
                nc.gpsimd.affine_select(
                    out=pt[:, col:col + P], in_=pt[:, col:col + P],
                    compare_op=mybir.AluOpType.is_ge, fill=0.0,
                    base=0, pattern=[[1, P]], channel_multiplier=-1,
                )

            # per-chunk pt tiles handed from s_exp to pv
            pts = {}

            def s_exp(ci):
                cs = slice(ci * CH, (ci + 1) * CH)
                lst = []
                # off-diagonal pairs: full 512-wide tiles, no mask
                for j in range(2 * ci):
                    ka, kb = 2 * j, 2 * j + 1
                    s2 = s_ps.tile([P, 2 * CH], F32, name="sT", tag="sT")
                    nc.tensor.matmul(s2[:, 0:CH], qk2[0:HD, ka * P:(ka + 1) * P],
                                     qk1[0:HD, cs], start=True, stop=True)
                    nc.tensor.matmul(s2[:, CH:2 * CH], qk1[HD:P, kb * P:(kb + 1) * P],
                                     qk2[HD:P, cs], start=True, stop=True)
                    pt = pwork.tile([P, 2 * CH], F16, name="pT", tag="pT")
                    nc.scalar.activation(pt[:], s2[:],
                                         mybir.ActivationFunctionType.Exp)
                    lst.append(pt)
                k0 = 4 * ci
                # diagonal pair A: d0 full at cols [0:512]; d1's live columns
                # (queries 128..511) at cols [512:896]. One exp over [0:896].
                s2 = s_ps.tile([P, 2 * CH], F32, name="sT", tag="sT")
                nc.tensor.matmul(s2[:, 0:CH], qk2[0:HD, k0 * P:(k0 + 1) * P],
                                 qk1[0:HD, cs], start=True, stop=True)
                nc.tensor.matmul(s2[:, CH:CH + 384],
                                 qk1[HD:P, (k0 + 1) * P:(k0 + 2) * P],
                                 qk2[HD:P, ci * CH + P:(ci + 1) * CH],
                                 start=True, stop=True)
                pt = pwork.tile([P, 2 * CH], F16, name="pT", tag="pT")
                nc.scalar.activation(pt[:, 0:CH + 384], s2[:, 0:CH + 384],
                                     mybir.ActivationFunctionType.Exp)
                _tri_mask(pt, 0)        # d0 triangle (queries 0..127)
                _tri_mask(pt, CH)       # d1 triangle (queries 128..255)
                lst.append(pt)
                # diagonal pair B: d2's live columns (queries 256..511) in
                # place at [256:512]; d3's (queries 384..511) at [512:640].
                # One exp over the contiguous live span [256:640].
                s2 = s_ps.tile([P, 2 * CH], F32, name="sT", tag="sT")
                nc.tensor.matmul(s2[:, 2 * P:CH],
                                 qk2[0:HD, (k0 + 2) * P:(k0 + 3) * P],
                                 qk1[0:HD, ci * CH + 2 * P:(ci + 1) * CH],
                                 start=True, stop=True)
                nc.tensor.matmul(s2[:, CH:CH + P],
                                 qk1[HD:P, (k0 + 3) * P:(k0 + 4) * P],
                                 qk2[HD:P, ci * CH + 3 * P:(ci + 1) * CH],
                                 start=True, stop=True)
                pt = pwork.tile([P, 2 * CH], F16, name="pT", tag="pT")
                nc.scalar.activation(pt[:, 2 * P:CH + P], s2[:, 2 * P:CH + P],
                                     mybir.ActivationFunctionType.Exp)
                _tri_mask(pt, 2 * P)    # d2 triangle (queries 256..383)
                _tri_mask(pt, CH)       # d3 triangle (queries 384..511)
                lst.append(pt)
                pts[ci] = lst

            def pv(ci):
                cs = slice(ci * CH, (ci + 1) * CH)
                lst = pts.pop(ci)
                ops = o_ps.tile([HD + 1, CH], F32, name="oacc", tag="oacc")
                for j in range(2 * ci):
                    ka, kb = 2 * j, 2 * j + 1
                    pt = lst[j]
                    nc.tensor.matmul(ops[:], v_aug[:, ka, :], pt[:, 0:CH],
                                     start=(j == 0), stop=False)
                    nc.tensor.matmul(ops[:], v_aug[:, kb, :], pt[:, CH:2 * CH],
                                     start=False, stop=False)
                k0 = 4 * ci
                ptA, ptB = lst[2 * ci], lst[2 * ci + 1]
                first = (ci == 0)
                # d0 covers all 512 columns; later tiles touch only their
                # live suffix (per-element has_written handles first-write).
                nc.tensor.matmul(ops[:], v_aug[:, k0, :], ptA[:, 0:CH],
                                 start=first, stop=False)
                nc.tensor.matmul(ops[:, P:CH], v_aug[:, k0 + 1, :],
                                 ptA[:, CH:CH + 384], start=False, stop=False)
                nc.tensor.matmul(ops[:, 2 * P:CH], v_aug[:, k0 + 2, :],
                                 ptB[:, 2 * P:CH], start=False, stop=False)
                nc.tensor.matmul(ops[:, 3 * P:CH], v_aug[:, k0 + 3, :],
                                 ptB[:, CH:CH + P], start=False, stop=True)
                osb = owork.tile([HD + 1, CH], F32, name="osb", tag="osb")
                nc.vector.tensor_copy(osb[:], ops[:])
                nc.sync.dma_start(out=outT[:, cs], in_=osb[:])

            # Emission order = scheduler priority, in bands: QK projections
            # and S/exp/mask (the ACT metronome's supply line) first, then
            # PVs (drain pt slots), then V projections+transposes last —
            # they have slack until their pv() and fill PE gaps, gated only
            # by data deps.
            qk_chunk(0)
            s_exp(0)
            qk_chunk(1)
            s_exp(1)
            qk_chunk(2)
            s_exp(2)
            qk_chunk(3)
            s_exp(3)
            v_pair(0, 1)
            pv(0)
            pv(1)
            v_pair(2, 3)
            pv(2)
            pv(3)

    if legalize:
        _legalize_waits(nc, mybir)
    return nc


def _legalize_waits(nc, mybir):
    """Split multi-wait instructions for the XLA-route walrus codegen.

    The TPB EVENTS struct holds one semaphore wait per instruction and this
    pipeline's codegen refuses >1. Hoist extra waits onto standalone
    EventSemaphore instructions on the same engine queue right before the
    instruction - semantically identical, the queue stalls there.
    """
    n = 0
    for f in nc.m.functions:
        for b in f.blocks:
            out = []
            changed = False
            for inst in b.instructions:
                si = inst.sync_info
                waits = list(si.on_wait) if si is not None and si.on_wait else []
                if len(waits) > 1:
                    changed = True
                    for w in waits[:-1]:
                        n += 1
                        out.append(mybir.InstEventSemaphore(
                            name=f"waitfix{n}_{inst.name}",
                            engine=inst.engine,
                            sync_info=mybir.SyncInfo(on_wait=[w], on_update=[]),
                        ))
                    inst.sync_info = mybir.SyncInfo(
                        on_wait=waits[-1:],
                        on_update=list(si.on_update or []),
                    )
                out.append(inst)
            if changed:
                b.instructions = out
    return n


def kernel(x, wq, bq, wk, bk, wv, bv):
    global LAST_RESULTS
    import os
    os.environ.setdefault("JAX_PLATFORMS", "")
    from concourse.bass_utils import run_bass_kernel_spmd

    x = np.asarray(x, dtype=np.float32)
    s = np.float32(1.0 / np.sqrt(HD))
    wq_s = np.asarray(wq, np.float32) * s
    wk_f = np.asarray(wk, np.float32)
    # (D, 128) -> (128, DT*128): partition p holds rows {n*128+p}
    w1 = np.concatenate([wq_s, wk_f], 0).T.astype(np.float16)  # (D, P)
    w1 = w1.reshape(DT, P, P).transpose(1, 0, 2).reshape(P, DT * P)
    wv_c = np.asarray(wv, np.float32).T.astype(np.float16)     # (D, HD)
    wv_c = wv_c.reshape(DT, P, HD).transpose(1, 0, 2).reshape(P, DT * HD)
    b1 = np.concatenate([np.asarray(bq, np.float32) * s,
                         np.asarray(bk, np.float32)]).reshape(P, 1)
    bv_f = np.asarray(bv, np.float32)
    bv_c = np.concatenate([bv_f, bv_f]).reshape(P, 1)
    wb = np.ascontiguousarray(np.concatenate(
        [w1, wv_c, b1.astype(np.float16), bv_c.astype(np.float16)], axis=1))
    # x (B,T,D) -> xT (B, P, NCH*DT*CH): xT[b,p,(ci,n,t)] = x[b, ci*CH+t, n*P+p]
    xT = np.swapaxes(x, 1, 2).astype(np.float16)               # (B, D, T)
    xT = xT.reshape(B, DT, P, NCH, CH)                         # D=(n,p), T=(ci,t)
    xT = np.ascontiguousarray(
        xT.transpose(0, 2, 3, 1, 4).reshape(B, P, NCH * DT * CH))

    nc = _build_module()
    in_maps = [{"xT": xT[b], "wb": wb} for b in range(B)]
    res = None
    for attempt in range(3):
        try:
            res = run_bass_kernel_spmd(nc, in_maps, core_ids=list(range(B)))
            break
        except Exception:
            # transient device wedges (NRT_EXEC_UNIT_UNRECOVERABLE) happen;
            # rebuild the module and retry on a clean execution
            if attempt == 2:
                raise
            nc = _build_module()
    LAST_RESULTS = res

    out = np.empty((B, T, HD), dtype=np.float32)
    for b in range(B):
        oT = res.results[b]["outT"]  # (65, T): rows 0..63 = O^T, row 64 = denom
        out[b] = (oT[:HD] / oT[HD:HD + 1]).T
    return out


# revision 40
# speedup vs baseline: 1.1093x; 1.1093x over previous
"""Trainium2 Bass kernel for a single causal attention head.

Problem: x:(8,2048,1024) f32, per-head projections wq/wk/wv:(64,1024),
biases (64,). Output: softmax(causal(q k^T / sqrt(64))) @ v : (8,2048,64).

Strategy:
  - Data-parallel: batch b -> core b (8 cores, 1 batch each).
  - Host prep: x[b] transposed to xT:(1024,2048) fp16 (contraction dim D on
    SBUF partitions); Q/K weights shipped stacked as [wq|wk] (fp16, 1/sqrt(64)
    folded into wq).
  - Device (per core):
      * Input DMA split across both HWDGE rings: x quarter-chunks on the sync
        ring, weights/biases on the scalar ring, so xq0 lands ~5us earlier
        than a single serialized ring.
      * qk1 = [wq|wk]^T.T @ xT: rows 0-63 = Q^T, rows 64-127 = K^T (PSUM
        accumulate over 8 d-tiles, fp16 matmuls, N=512 chunks).
      * qk2 = half-swapped copy of qk1: K^T on rows 0-63, Q^T on rows 64-127.
        Both orders exist on both partition halves -> scores for TWO k-tiles
        run concurrently via PE row packing.
      * vT transposed back to (T,64) tiles via PE transpose, augmented with a
        ones column (softmax denominator rides along the PV matmul).
      * S^T[j,i] = sum_h K^T[h,j] Q^T[h,i] transposed-scores layout; P^T =
        exp(S^T) on ACT; causal mask = GPSIMD affine_select zeroing P^T in
        the 128x128 diagonal triangle blocks only (fully-masked column
        prefixes of diagonal tiles are skipped by S/exp/PV instead of
        computed-then-zeroed).
      * O^T_aug[65, T] accumulated in PSUM over k-tiles; row 64 = sum_j P^T.
        PSUM accumulate is per-element (has_written), so later k-tiles may
        touch columns the first matmul didn't.
      * causal skip: k-tiles entirely above the diagonal never computed;
        diagonal tiles compute only their live column suffix.
      * emission order interleaves chunk ci+1 projections ahead of chunk ci
        attention so the scalar engine (exp = the metronome) never starves.
  - Host post: out[b] = (O^T[0:64] / O^T[64:65]).T  (softmax normalization).
"""

import numpy as np

B, T, D, HD = 8, 2048, 1024, 64
P = 128          # SBUF partitions
CH = 512         # q-chunk (matmul moving dim)
NCH = T // CH    # 4
DT = D // P      # 8 d-tiles
NKT = T // P     # 16 k-tiles

LAST_RESULTS = None      # BassKernelResults of the most recent run (for test.py)


def _build_module(legalize=True):
    import concourse.bass as bass
    import concourse.mybir as mybir
    from concourse.tile import TileContext

    from concourse.masks import make_identity
    F32 = mybir.dt.float32
    F16 = mybir.dt.float16

    nc = bass.Bass("TRN2", target_bir_lowering=True)

    # All inputs host-pre-arranged to the exact SBUF layout so every DMA is
    # contiguous per partition (128 big descriptors, not 1024 tiny gathers —
    # the rearrange-in-DMA variant was descriptor-bound and took ~10us).
    xT = nc.dram_tensor("xT", (P, NCH * DT * CH), F16, kind="ExternalInput")
    # one weight blob = [w1 | wv | b1 | bv] per partition -> a single DMA
    # (each separate transfer costs ~2us completion receipt on its ring)
    WQOF = DT * P              # w1 cols
    WVOF = WQOF + DT * HD      # wv cols start at WQOF
    BOF = WVOF                 # b1 at BOF, bv at BOF+1
    wb = nc.dram_tensor("wb", (P, BOF + 2), F16, kind="ExternalInput")
    outT = nc.dram_tensor("outT", (HD + 1, T), F32, kind="ExternalOutput")

    with TileContext(nc) as tc:
        with (
            tc.tile_pool(name="const", bufs=1) as const,
            tc.tile_pool(name="acts", bufs=1) as acts,
            tc.tile_pool(name="proj_ps", bufs=1, space="PSUM") as proj_ps,
            tc.tile_pool(name="tr_ps", bufs=1, space="PSUM") as tr_ps,
            tc.tile_pool(name="s_ps", bufs=2, space="PSUM") as s_ps,
            tc.tile_pool(name="o_ps", bufs=2, space="PSUM") as o_ps,
            tc.tile_pool(name="pwork", bufs=20) as pwork,
            tc.tile_pool(name="owork", bufs=2) as owork,
        ):
            ident = const.tile([P, P], F32, name="ident")
            make_identity(nc, ident)

            # ---- weights/biases on the GPSIMD (SWDGE) ring so they drain in
            # parallel with the x quarters on the sync HWDGE ring. (The
            # scalar HWDGE ring is NOT used for input: a DIRECT2D occupies
            # the ACT queue for the whole transfer and would stall exps.)
            # Input DMA runs at the shared-HBM roofline (~310 GB/s/core with
            # all 8 cores loading at once) - 4.5MB takes ~14us no matter how
            # it's split. The blob rides the scalar ring with xq1 behind it
            # (that queue must drain before the first real exp ~17us); the
            # other chunks serialize on the sync ring in consumption order.
            wb_sb = const.tile([P, BOF + 2], F16, name="wb_sb")
            nc.scalar.dma_start(out=wb_sb[:], in_=wb[:, :])
            bias = const.tile([P, 2], F32, name="bias")
            nc.vector.tensor_copy(bias[:], wb_sb[:, BOF:BOF + 2])
            b1_sb = bias[:, 0:1]
            bv_sb = bias[:, 1:2]

            xq = []
            for ci in range(NCH):
                t = const.tile([P, DT * CH], F16, name=f"xq{ci}")
                nc.sync.dma_start(
                    out=t[:], in_=xT[:, ci * DT * CH:(ci + 1) * DT * CH])
                xq.append(t)

            wscr = const.tile([P, 2 * P], F16, name="wscr")
            nc.vector.memset(wscr[:], 0.0)
            # dummy activation up front: walrus inserts the ~1.3us
            # ACT_TABLE_LOAD before the first exp; issuing it at t~0 hides
            # the load under the input DMA instead of the first real exp.
            edum = const.tile([P, 1], F16, name="edum")
            nc.scalar.activation(edum[:], wscr[:, 0:1],
                                 mybir.ActivationFunctionType.Exp)
            # HAM warm-up: short throwaway matmuls keep the PE active through
            # its 3.4us activity window so real matmuls run at the full
            # 2.4 GHz clock. From the bufs=2 pool so they pipeline instead of
            # serializing on slot release; sized to end about when xq0 lands.
            for wu in range(16):
                pswu = s_ps.tile([P, 2 * P], F32, name="warm", tag="sT")
                nc.tensor.matmul(pswu[:], wscr[:, 0:P], wscr[:],
                                 start=True, stop=True)

            # ---- activations ----
            qk1 = acts.tile([P, T], F16, name="qk1")
            qk2 = acts.tile([P, T], F16, name="qk2")
            # vT2: V^T for a chunk PAIR stacked on partition halves (rows
            # 0-63 = even chunk, 64-127 = odd chunk) -> one 128x128 PE
            # transpose yields TWO v_aug k-tiles.
            vT2 = acts.tile([P, T // 2], F32, name="vT2")
            v_aug = acts.tile([P, NKT, HD + 1], F16, name="v_aug")
            nc.vector.memset(v_aug[:, :, HD], 1.0)

            def qk_chunk(ci):
                cs = slice(ci * CH, (ci + 1) * CH)
                rhs = xq[ci]
                ps = proj_ps.tile([P, CH], F32, name="proj", tag="proj")
                for d in range(DT):
                    nc.tensor.matmul(ps[:], wb_sb[:, d * P:(d + 1) * P],
                                     rhs[:, d * CH:(d + 1) * CH],
                                     start=(d == 0), stop=(d == DT - 1))
                nc.vector.tensor_scalar_add(qk1[:, cs], ps[:], b1_sb)
                # half-swapped copy: qk2 = [K^T; Q^T]. 64-partition DVE ops
                # read any aligned src half and write either dest half.
                nc.vector.tensor_copy(qk2[0:HD, cs], qk1[HD:P, cs])
                nc.vector.tensor_copy(qk2[HD:P, cs], qk1[0:HD, cs])

            def v_pair(ca, cb):
                # V projections for two chunks col-packed: chunk ca on array
                # columns 0-63, chunk cb on columns 64-127 -> the matmul pairs
                # overlap in the PE array; outputs land in disjoint halves of
                # one PSUM bank. The stacked vT2 halves then transpose two
                # k-tiles per 128x128 PE transpose.
                off = (ca // 2) * CH
                psv = proj_ps.tile([P, CH], F32, name="projv", tag="proj")
                for d in range(DT):
                    wvd = wb_sb[:, WQOF + d * HD:WQOF + (d + 1) * HD]
                    nc.tensor.matmul(psv[0:HD, :], wvd,
                                     xq[ca][:, d * CH:(d + 1) * CH],
                                     start=(d == 0), stop=(d == DT - 1))
                    nc.tensor.matmul(psv[HD:P, :], wvd,
                                     xq[cb][:, d * CH:(d + 1) * CH],
                                     start=(d == 0), stop=(d == DT - 1))
                nc.vector.tensor_scalar_add(
                    vT2[0:HD, off:off + CH], psv[0:HD, :], bv_sb[0:HD])
                nc.vector.tensor_scalar_add(
                    vT2[HD:P, off:off + CH], psv[HD:P, :], bv_sb[HD:P])
                for i in range(4):
                    tp = tr_ps.tile([P, 2, HD], F32, name="vtr", tag="vtr")
                    nc.tensor.transpose(tp[:, :, :],
                                        vT2[:, off + i * P:off + (i + 1) * P],
                                        ident[:, :])
                    nc.vector.tensor_copy(
                        v_aug[:, 4 * ca + i:4 * cb + i + 1:4, 0:HD], tp[:, :, :])

            def _tri_mask(pt, col):
                # zero the strictly-above-diagonal half of a 128x128 block
                # whose query offset equals its key offset (iota = c - j).
                nc.gpsimd.affine_select(
                    out=pt[:, col:col + P], in_=pt[:, col:col + P],
                    compare_op=mybir.AluOpType.is_ge, fill=0.0,
                    base=0, pattern=[[1, P]], channel_multiplier=-1,
                )

            # per-chunk pt tiles handed from s_exp to pv
            pts = {}

            def s_exp(ci):
                cs = slice(ci * CH, (ci + 1) * CH)
                lst = []
                # off-diagonal pairs: full 512-wide tiles, no mask
                for j in range(2 * ci):
                    ka, kb = 2 * j, 2 * j + 1
                    s2 = s_ps.tile([P, 2 * CH], F32, name="sT", tag="sT")
                    nc.tensor.matmul(s2[:, 0:CH], qk2[0:HD, ka * P:(ka + 1) * P],
                                     qk1[0:HD, cs], start=True, stop=True)
                    nc.tensor.matmul(s2[:, CH:2 * CH], qk1[HD:P, kb * P:(kb + 1) * P],
                                     qk2[HD:P, cs], start=True, stop=True)
                    pt = pwork.tile([P, 2 * CH], F16, name="pT", tag="pT")
                    nc.scalar.activation(pt[:], s2[:],
                                         mybir.ActivationFunctionType.Exp)
                    lst.append(pt)
                k0 = 4 * ci
                # diagonal pair A: d0 full at cols [0:512]; d1's live columns
                # (queries 128..511) at cols [512:896]. One exp over [0:896].
                s2 = s_ps.tile([P, 2 * CH], F32, name="sT", tag="sT")
                nc.tensor.matmul(s2[:, 0:CH], qk2[0:HD, k0 * P:(k0 + 1) * P],
                                 qk1[0:HD, cs], start=True, stop=True)
                nc.tensor.matmul(s2[:, CH:CH + 384],
                                 qk1[HD:P, (k0 + 1) * P:(k0 + 2) * P],
                                 qk2[HD:P, ci * CH + P:(ci + 1) * CH],
                                 start=True, stop=True)
                pt = pwork.tile([P, 2 * CH], F16, name="pT", tag="pT")
                nc.scalar.activation(pt[:, 0:CH + 384], s2[:, 0:CH + 384],
                                     mybir.ActivationFunctionType.Exp)
                _tri_mask(pt, 0)        # d0 triangle (queries 0..127)
                _tri_mask(pt, CH)       # d1 triangle (queries 128..255)
                lst.append(pt)
                # diagonal pair B: d2's live columns (queries 256..511) in
                # place at [256:512]; d3's (queries 384..511) at [512:640].
                # One exp over the contiguous live span [256:640].
                s2 = s_ps.tile([P, 2 * CH], F32, name="sT", tag="sT")
                nc.tensor.matmul(s2[:, 2 * P:CH],
                                 qk2[0:HD, (k0 + 2) * P:(k0 + 3) * P],
                                 qk1[0:HD, ci * CH + 2 * P:(ci + 1) * CH],
                                 start=True, stop=True)
                nc.tensor.matmul(s2[:, CH:CH + P],
                                 qk1[HD:P, (k0 + 3) * P:(k0 + 4) * P],
                                 qk2[HD:P, ci * CH + 3 * P:(ci + 1) * CH],
                                 start=True, stop=True)
                pt = pwork.tile([P, 2 * CH], F16, name="pT", tag="pT")
                nc.scalar.activation(pt[:, 2 * P:CH + P], s2[:, 2 * P:CH + P],
                                     mybir.ActivationFunctionType.Exp)
                _tri_mask(pt, 2 * P)    # d2 triangle (queries 256..383)
                _tri_mask(pt, CH)       # d3 triangle (queries 384..511)
                lst.append(pt)
                pts[ci] = lst

            def pv(ci):
                cs = slice(ci * CH, (ci + 1) * CH)
                lst = pts.pop(ci)
                ops = o_ps.tile([HD + 1, CH], F32, name="oacc", tag="oacc")
                for j in range(2 * ci):
                    ka, kb = 2 * j, 2 * j + 1
                    pt = lst[j]
                    nc.tensor.matmul(ops[:], v_aug[:, ka, :], pt[:, 0:CH],
                                     start=(j == 0), stop=False)
                    nc.tensor.matmul(ops[:], v_aug[:, kb, :], pt[:, CH:2 * CH],
                                     start=False, stop=False)
                k0 = 4 * ci
                ptA, ptB = lst[2 * ci], lst[2 * ci + 1]
                first = (ci == 0)
                # d0 covers all 512 columns; later tiles touch only their
                # live suffix (per-element has_written handles first-write).
                nc.tensor.matmul(ops[:], v_aug[:, k0, :], ptA[:, 0:CH],
                                 start=first, stop=False)
                nc.tensor.matmul(ops[:, P:CH], v_aug[:, k0 + 1, :],
                                 ptA[:, CH:CH + 384], start=False, stop=False)
                nc.tensor.matmul(ops[:, 2 * P:CH], v_aug[:, k0 + 2, :],
                                 ptB[:, 2 * P:CH], start=False, stop=False)
                nc.tensor.matmul(ops[:, 3 * P:CH], v_aug[:, k0 + 3, :],
                                 ptB[:, CH:CH + P], start=False, stop=True)
                osb = owork.tile([HD + 1, CH], F32, name="osb", tag="osb")
                nc.vector.tensor_copy(osb[:], ops[:])
                nc.sync.dma_start(out=outT[:, cs], in_=osb[:])

            # Emission order = scheduler priority, in bands: QK projections
            # and S/exp/mask (the ACT metronome's supply line) first, then
            # PVs (drain pt slots), then V projections+transposes last —
            # they have slack until their pv() and fill PE gaps, gated only
            # by data deps.
            qk_chunk(0)
            s_exp(0)
            qk_chunk(1)
            s_exp(1)
            qk_chunk(2)
            s_exp(2)
            qk_chunk(3)
            s_exp(3)
            v_pair(0, 1)
            pv(0)
            pv(1)
            v_pair(2, 3)
            pv(2)
            pv(3)

    if legalize:
        _legalize_waits(nc, mybir)
    return nc


def _legalize_waits(nc, mybir):
    """Split multi-wait instructions for the XLA-route walrus codegen.

    The TPB EVENTS struct holds one semaphore wait per instruction and this
    pipeline's codegen refuses >1. Hoist extra waits onto standalone
    EventSemaphore instructions on the same engine queue right before the
    instruction - semantically identical, the queue stalls there.
    """
    n = 0
    for f in nc.m.functions:
        for b in f.blocks:
            out = []
            changed = False
            for inst in b.instructions:
                si = inst.sync_info
                waits = list(si.on_wait) if si is not None and si.on_wait else []
                if len(waits) > 1:
                    changed = True
                    for w in waits[:-1]:
                        n += 1
                        out.append(mybir.InstEventSemaphore(
                            name=f"waitfix{n}_{inst.name}",
                            engine=inst.engine,
                            sync_info=mybir.SyncInfo(on_wait=[w], on_update=[]),
                        ))
                    inst.sync_info = mybir.SyncInfo(
                        on_wait=waits[-1:],
                        on_update=list(si.on_update or []),
                    )
                out.append(inst)
            if changed:
                b.instructions = out
    return n


def kernel(x, wq, bq, wk, bk, wv, bv):
    global LAST_RESULTS
    import os
    os.environ.setdefault("JAX_PLATFORMS", "")
    from concourse.bass_utils import run_bass_kernel_spmd

    x = np.asarray(x, dtype=np.float32)
    s = np.float32(1.0 / np.sqrt(HD))
    wq_s = np.asarray(wq, np.float32) * s
    wk_f = np.asarray(wk, np.float32)
    # (D, 128) -> (128, DT*128): partition p holds rows {n*128+p}
    w1 = np.concatenate([wq_s, wk_f], 0).T.astype(np.float16)  # (D, P)
    w1 = w1.reshape(DT, P, P).transpose(1, 0, 2).reshape(P, DT * P)
    wv_c = np.asarray(wv, np.float32).T.astype(np.float16)     # (D, HD)
    wv_c = wv_c.reshape(DT, P, HD).transpose(1, 0, 2).reshape(P, DT * HD)
    b1 = np.concatenate([np.asarray(bq, np.float32) * s,
                         np.asarray(bk, np.float32)]).reshape(P, 1)
    bv_f = np.asarray(bv, np.float32)
    bv_c = np.concatenate([bv_f, bv_f]).reshape(P, 1)
    wb = np.ascontiguousarray(np.concatenate(
        [w1, wv_c, b1.astype(np.float16), bv_c.astype(np.float16)], axis=1))
    # x (B,T,D) -> xT (B, P, NCH*DT*CH): xT[b,p,(ci,n,t)] = x[b, ci*CH+t, n*P+p]
    xT = np.swapaxes(x, 1, 2).astype(np.float16)               # (B, D, T)
    xT = xT.reshape(B, DT, P, NCH, CH)                         # D=(n,p), T=(ci,t)
    xT = np.ascontiguousarray(
        xT.transpose(0, 2, 3, 1, 4).reshape(B, P, NCH * DT * CH))

    nc = _build_module()
    in_maps = [{"xT": xT[b], "wb": wb} for b in range(B)]
    res = None
    for attempt in range(3):
        try:
            res = run_bass_kernel_spmd(nc, in_maps, core_ids=list(range(B)))
            break
        except Exception:
            # transient device wedges (NRT_EXEC_UNIT_UNRECOVERABLE) happen;
            # rebuild the module and retry on a clean execution
            if attempt == 2:
                raise
            nc = _build_module()
    LAST_RESULTS = res

    out = np.empty((B, T, HD), dtype=np.float32)
    for b in range(B):
        oT = res.results[b]["outT"]  # (65, T): rows 0..63 = O^T, row 64 = denom
        out[b] = (oT[:HD] / oT[HD:HD + 1]).T
    return out


# revision 47
# speedup vs baseline: 1.1557x; 1.0418x over previous
"""Trainium2 Bass kernel for a single causal attention head.

Problem: x:(8,2048,1024) f32, per-head projections wq/wk/wv:(64,1024),
biases (64,). Output: softmax(causal(q k^T / sqrt(64))) @ v : (8,2048,64).

Strategy:
  - Data-parallel: batch b -> core b (8 cores, 1 batch each).
  - Host prep: x[b] transposed to xT:(1024,2048) fp16 (contraction dim D on
    SBUF partitions); Q/K weights shipped stacked as [wq|wk] (fp16, 1/sqrt(64)
    folded into wq).
  - Device (per core):
      * Input DMA split across both HWDGE rings: x quarter-chunks on the sync
        ring, weights/biases on the scalar ring, so xq0 lands ~5us earlier
        than a single serialized ring.
      * qk1 = [wq|wk]^T.T @ xT: rows 0-63 = Q^T, rows 64-127 = K^T (PSUM
        accumulate over 8 d-tiles, fp16 matmuls, N=512 chunks).
      * qk2 = half-swapped copy of qk1: K^T on rows 0-63, Q^T on rows 64-127.
        Both orders exist on both partition halves -> scores for TWO k-tiles
        run concurrently via PE row packing.
      * vT transposed back to (T,64) tiles via PE transpose, augmented with a
        ones column (softmax denominator rides along the PV matmul).
      * S^T[j,i] = sum_h K^T[h,j] Q^T[h,i] transposed-scores layout; P^T =
        exp(S^T) on ACT; causal mask = GPSIMD affine_select zeroing P^T in
        the 128x128 diagonal triangle blocks only (fully-masked column
        prefixes of diagonal tiles are skipped by S/exp/PV instead of
        computed-then-zeroed).
      * O^T_aug[65, T] accumulated in PSUM over k-tiles; row 64 = sum_j P^T.
        PSUM accumulate is per-element (has_written), so later k-tiles may
        touch columns the first matmul didn't.
      * causal skip: k-tiles entirely above the diagonal never computed;
        diagonal tiles compute only their live column suffix.
      * emission order interleaves chunk ci+1 projections ahead of chunk ci
        attention so the scalar engine (exp = the metronome) never starves.
  - Host post: out[b] = (O^T[0:64] / O^T[64:65]).T  (softmax normalization).
"""

import numpy as np

B, T, D, HD = 8, 2048, 1024, 64
P = 128          # SBUF partitions
CH = 512         # q-chunk (matmul moving dim)
NCH = T // CH    # 4
DT = D // P      # 8 d-tiles
NKT = T // P     # 16 k-tiles

LAST_RESULTS = None      # BassKernelResults of the most recent run (for test.py)


def _build_module(legalize=True):
    import concourse.bass as bass
    import concourse.mybir as mybir
    from concourse.tile import TileContext

    from concourse.masks import make_identity
    F32 = mybir.dt.float32
    F16 = mybir.dt.float16

    nc = bass.Bass("TRN2", target_bir_lowering=True)

    # All inputs host-pre-arranged to the exact SBUF layout so every DMA is
    # contiguous per partition (128 big descriptors, not 1024 tiny gathers —
    # the rearrange-in-DMA variant was descriptor-bound and took ~10us).
    # First transfer = [w1 | wv | b1 | bv | x-chunk0] per partition: ONE DMA
    # carrying exactly what the first projection needs, at full ring
    # bandwidth with a single completion receipt. Later x chunks follow
    # serially on the same ring in consumption order.
    WQOF = DT * P              # w1 cols
    BOF = WQOF + DT * HD       # b1 at BOF, bv at BOF+1 (wv starts at WQOF)
    XOF = BOF + 2              # chunk-0 x cols
    wb = nc.dram_tensor("wb", (P, XOF + DT * CH), F16, kind="ExternalInput")
    xT = nc.dram_tensor("xT", (P, (NCH - 1) * DT * CH), F16,
                        kind="ExternalInput")
    outT = nc.dram_tensor("outT", (HD + 1, T), F32, kind="ExternalOutput")

    with TileContext(nc) as tc:
        with (
            tc.tile_pool(name="const", bufs=1) as const,
            tc.tile_pool(name="acts", bufs=1) as acts,
            tc.tile_pool(name="proj_ps", bufs=1, space="PSUM") as proj_ps,
            tc.tile_pool(name="tr_ps", bufs=1, space="PSUM") as tr_ps,
            tc.tile_pool(name="s_ps", bufs=2, space="PSUM") as s_ps,
            tc.tile_pool(name="o_ps", bufs=2, space="PSUM") as o_ps,
            tc.tile_pool(name="pwork", bufs=20) as pwork,
            tc.tile_pool(name="owork", bufs=2) as owork,
        ):
            ident = const.tile([P, P], F32, name="ident")
            make_identity(nc, ident)

            # ---- weights/biases on the GPSIMD (SWDGE) ring so they drain in
            # parallel with the x quarters on the sync HWDGE ring. (The
            # scalar HWDGE ring is NOT used for input: a DIRECT2D occupies
            # the ACT queue for the whole transfer and would stall exps.)
            # Input DMA runs at the shared-HBM roofline (~310 GB/s/core with
            # all 8 cores loading at once) - 4.5MB takes ~14us no matter how
            # it's split. The blob rides the scalar ring with xq1 behind it
            # (that queue must drain before the first real exp ~17us); the
            # other chunks serialize on the sync ring in consumption order.
            wb_sb = const.tile([P, XOF + DT * CH], F16, name="wb_sb")
            nc.sync.dma_start(out=wb_sb[:], in_=wb[:, :])
            bias = const.tile([P, 2], F32, name="bias")
            nc.vector.tensor_copy(bias[:], wb_sb[:, BOF:BOF + 2])
            b1_sb = bias[:, 0:1]
            bv_sb = bias[:, 1:2]

            # xq[ci] = (tile, column offset of that chunk's x block)
            xq = [(wb_sb, XOF)]
            for ci in range(1, NCH):
                t = const.tile([P, DT * CH], F16, name=f"xq{ci}")
                nc.sync.dma_start(
                    out=t[:], in_=xT[:, (ci - 1) * DT * CH:ci * DT * CH])
                xq.append((t, 0))

            wscr = const.tile([P, 2 * P], F16, name="wscr")
            nc.vector.memset(wscr[:], 0.0)
            # dummy activation up front: walrus inserts the ~1.3us
            # ACT_TABLE_LOAD before the first exp; issuing it at t~0 hides
            # the load under the input DMA instead of the first real exp.
            edum = const.tile([P, 1], F16, name="edum")
            nc.scalar.activation(edum[:], wscr[:, 0:1],
                                 mybir.ActivationFunctionType.Exp)
            # HAM warm-up: short throwaway matmuls keep the PE active through
            # its 3.4us activity window so real matmuls run at the full
            # 2.4 GHz clock. From the bufs=2 pool so they pipeline instead of
            # serializing on slot release; sized to end about when xq0 lands.
            for wu in range(16):
                pswu = s_ps.tile([P, 2 * P], F32, name="warm", tag="sT")
                nc.tensor.matmul(pswu[:], wscr[:, 0:P], wscr[:],
                                 start=True, stop=True)

            # ---- activations ----
            qk1 = acts.tile([P, T], F16, name="qk1")
            qk2 = acts.tile([P, T], F16, name="qk2")
            # vT2: V^T for a chunk PAIR stacked on partition halves (rows
            # 0-63 = even chunk, 64-127 = odd chunk) -> one 128x128 PE
            # transpose yields TWO v_aug k-tiles.
            vT2 = acts.tile([P, T // 2], F32, name="vT2")
            v_aug = acts.tile([P, NKT, HD + 1], F16, name="v_aug")
            nc.vector.memset(v_aug[:, :, HD], 1.0)

            def qk_chunk(ci):
                cs = slice(ci * CH, (ci + 1) * CH)
                rhs, ro = xq[ci]
                ps = proj_ps.tile([P, CH], F32, name="proj", tag="proj")
                for d in range(DT):
                    nc.tensor.matmul(ps[:], wb_sb[:, d * P:(d + 1) * P],
                                     rhs[:, ro + d * CH:ro + (d + 1) * CH],
                                     start=(d == 0), stop=(d == DT - 1))
                nc.vector.tensor_scalar_add(qk1[:, cs], ps[:], b1_sb)
                # half-swapped copy: qk2 = [K^T; Q^T]. 64-partition DVE ops
                # read any aligned src half and write either dest half.
                nc.vector.tensor_copy(qk2[0:HD, cs], qk1[HD:P, cs])
                nc.vector.tensor_copy(qk2[HD:P, cs], qk1[0:HD, cs])

            def v_pair(ca, cb):
                # V projections for two chunks col-packed: chunk ca on array
                # columns 0-63, chunk cb on columns 64-127 -> the matmul pairs
                # overlap in the PE array; outputs land in disjoint halves of
                # one PSUM bank. The stacked vT2 halves then transpose two
                # k-tiles per 128x128 PE transpose.
                off = (ca // 2) * CH
                psv = proj_ps.tile([P, CH], F32, name="projv", tag="proj")
                (ta, oa), (tb, ob) = xq[ca], xq[cb]
                for d in range(DT):
                    wvd = wb_sb[:, WQOF + d * HD:WQOF + (d + 1) * HD]
                    nc.tensor.matmul(psv[0:HD, :], wvd,
                                     ta[:, oa + d * CH:oa + (d + 1) * CH],
                                     start=(d == 0), stop=(d == DT - 1))
                    nc.tensor.matmul(psv[HD:P, :], wvd,
                                     tb[:, ob + d * CH:ob + (d + 1) * CH],
                                     start=(d == 0), stop=(d == DT - 1))
                nc.vector.tensor_scalar_add(
                    vT2[0:HD, off:off + CH], psv[0:HD, :], bv_sb[0:HD])
                nc.vector.tensor_scalar_add(
                    vT2[HD:P, off:off + CH], psv[HD:P, :], bv_sb[HD:P])
                for i in range(4):
                    tp = tr_ps.tile([P, 2, HD], F32, name="vtr", tag="vtr")
                    nc.tensor.transpose(tp[:, :, :],
                                        vT2[:, off + i * P:off + (i + 1) * P],
                                        ident[:, :])
                    nc.vector.tensor_copy(
                        v_aug[:, 4 * ca + i:4 * cb + i + 1:4, 0:HD], tp[:, :, :])

            def _tri_mask(pt, col):
                # zero the strictly-above-diagonal half of a 128x128 block
                # whose query offset equals its key offset (iota = c - j).
                nc.gpsimd.affine_select(
                    out=pt[:, col:col + P], in_=pt[:, col:col + P],
                    compare_op=mybir.AluOpType.is_ge, fill=0.0,
                    base=0, pattern=[[1, P]], channel_multiplier=-1,
                )

            # per-chunk pt tiles handed from s_exp to pv
            pts = {}

            def s_exp(ci):
                cs = slice(ci * CH, (ci + 1) * CH)
                lst = []
                # off-diagonal pairs: full 512-wide tiles, no mask
                for j in range(2 * ci):
                    ka, kb = 2 * j, 2 * j + 1
                    s2 = s_ps.tile([P, 2 * CH], F32, name="sT", tag="sT")
                    nc.tensor.matmul(s2[:, 0:CH], qk2[0:HD, ka * P:(ka + 1) * P],
                                     qk1[0:HD, cs], start=True, stop=True)
                    nc.tensor.matmul(s2[:, CH:2 * CH], qk1[HD:P, kb * P:(kb + 1) * P],
                                     qk2[HD:P, cs], start=True, stop=True)
                    pt = pwork.tile([P, 2 * CH], F16, name="pT", tag="pT")
                    nc.scalar.activation(pt[:], s2[:],
                                         mybir.ActivationFunctionType.Exp)
                    lst.append(pt)
                k0 = 4 * ci
                # diagonal pair A: d0 full at cols [0:512]; d1's live columns
                # (queries 128..511) at cols [512:896]. One exp over [0:896].
                s2 = s_ps.tile([P, 2 * CH], F32, name="sT", tag="sT")
                nc.tensor.matmul(s2[:, 0:CH], qk2[0:HD, k0 * P:(k0 + 1) * P],
                                 qk1[0:HD, cs], start=True, stop=True)
                nc.tensor.matmul(s2[:, CH:CH + 384],
                                 qk1[HD:P, (k0 + 1) * P:(k0 + 2) * P],
                                 qk2[HD:P, ci * CH + P:(ci + 1) * CH],
                                 start=True, stop=True)
                pt = pwork.tile([P, 2 * CH], F16, name="pT", tag="pT")
                nc.scalar.activation(pt[:, 0:CH + 384], s2[:, 0:CH + 384],
                                     mybir.ActivationFunctionType.Exp)
                _tri_mask(pt, 0)        # d0 triangle (queries 0..127)
                _tri_mask(pt, CH)       # d1 triangle (queries 128..255)
                lst.append(pt)
                # diagonal pair B: d2's live columns (queries 256..511) in
                # place at [256:512]; d3's (queries 384..511) at [512:640].
                # One exp over the contiguous live span [256:640].
                s2 = s_ps.tile([P, 2 * CH], F32, name="sT", tag="sT")
                nc.tensor.matmul(s2[:, 2 * P:CH],
                                 qk2[0:HD, (k0 + 2) * P:(k0 + 3) * P],
                                 qk1[0:HD, ci * CH + 2 * P:(ci + 1) * CH],
                                 start=True, stop=True)
                nc.tensor.matmul(s2[:, CH:CH + P],
                                 qk1[HD:P, (k0 + 3) * P:(k0 + 4) * P],
                                 qk2[HD:P, ci * CH + 3 * P:(ci + 1) * CH],
                                 start=True, stop=True)
                pt = pwork.tile([P, 2 * CH], F16, name="pT", tag="pT")
                nc.scalar.activation(pt[:, 2 * P:CH + P], s2[:, 2 * P:CH + P],
                                     mybir.ActivationFunctionType.Exp)
                _tri_mask(pt, 2 * P)    # d2 triangle (queries 256..383)
                _tri_mask(pt, CH)       # d3 triangle (queries 384..511)
                lst.append(pt)
                pts[ci] = lst

            def pv(ci):
                cs = slice(ci * CH, (ci + 1) * CH)
                lst = pts.pop(ci)
                ops = o_ps.tile([HD + 1, CH], F32, name="oacc", tag="oacc")
                for j in range(2 * ci):
                    ka, kb = 2 * j, 2 * j + 1
                    pt = lst[j]
                    nc.tensor.matmul(ops[:], v_aug[:, ka, :], pt[:, 0:CH],
                                     start=(j == 0), stop=False)
                    nc.tensor.matmul(ops[:], v_aug[:, kb, :], pt[:, CH:2 * CH],
                                     start=False, stop=False)
                k0 = 4 * ci
                ptA, ptB = lst[2 * ci], lst[2 * ci + 1]
                first = (ci == 0)
                # d0 covers all 512 columns; later tiles touch only their
                # live suffix (per-element has_written handles first-write).
                nc.tensor.matmul(ops[:], v_aug[:, k0, :], ptA[:, 0:CH],
                                 start=first, stop=False)
                nc.tensor.matmul(ops[:, P:CH], v_aug[:, k0 + 1, :],
                                 ptA[:, CH:CH + 384], start=False, stop=False)
                nc.tensor.matmul(ops[:, 2 * P:CH], v_aug[:, k0 + 2, :],
                                 ptB[:, 2 * P:CH], start=False, stop=False)
                nc.tensor.matmul(ops[:, 3 * P:CH], v_aug[:, k0 + 3, :],
                                 ptB[:, CH:CH + P], start=False, stop=True)
                osb = owork.tile([HD + 1, CH], F32, name="osb", tag="osb")
                nc.vector.tensor_copy(osb[:], ops[:])
                nc.sync.dma_start(out=outT[:, cs], in_=osb[:])

            # Emission order = scheduler priority, in bands: QK projections
            # and S/exp/mask (the ACT metronome's supply line) first, then
            # PVs (drain pt slots), then V projections+transposes last —
            # they have slack until their pv() and fill PE gaps, gated only
            # by data deps.
            qk_chunk(0)
            s_exp(0)
            qk_chunk(1)
            s_exp(1)
            qk_chunk(2)
            s_exp(2)
            qk_chunk(3)
            s_exp(3)
            v_pair(0, 1)
            pv(0)
            pv(1)
            v_pair(2, 3)
            pv(2)
            pv(3)

    if legalize:
        _legalize_waits(nc, mybir)
    return nc


def _legalize_waits(nc, mybir):
    """Split multi-wait instructions for the XLA-route walrus codegen.

    The TPB EVENTS struct holds one semaphore wait per instruction and this
    pipeline's codegen refuses >1. Hoist extra waits onto standalone
    EventSemaphore instructions on the same engine queue right before the
    instruction - semantically identical, the queue stalls there.
    """
    n = 0
    for f in nc.m.functions:
        for b in f.blocks:
            out = []
            changed = False
            for inst in b.instructions:
                si = inst.sync_info
                waits = list(si.on_wait) if si is not None and si.on_wait else []
                if len(waits) > 1:
                    changed = True
                    for w in waits[:-1]:
                        n += 1
                        out.append(mybir.InstEventSemaphore(
                            name=f"waitfix{n}_{inst.name}",
                            engine=inst.engine,
                            sync_info=mybir.SyncInfo(on_wait=[w], on_update=[]),
                        ))
                    inst.sync_info = mybir.SyncInfo(
                        on_wait=waits[-1:],
                        on_update=list(si.on_update or []),
                    )
                out.append(inst)
            if changed:
                b.instructions = out
    return n


def kernel(x, wq, bq, wk, bk, wv, bv):
    global LAST_RESULTS
    import os
    os.environ.setdefault("JAX_PLATFORMS", "")
    from concourse.bass_utils import run_bass_kernel_spmd

    x = np.asarray(x, dtype=np.float32)
    s = np.float32(1.0 / np.sqrt(HD))
    wq_s = np.asarray(wq, np.float32) * s
    wk_f = np.asarray(wk, np.float32)
    # (D, 128) -> (128, DT*128): partition p holds rows {n*128+p}
    w1 = np.concatenate([wq_s, wk_f], 0).T.astype(np.float16)  # (D, P)
    w1 = w1.reshape(DT, P, P).transpose(1, 0, 2).reshape(P, DT * P)
    wv_c = np.asarray(wv, np.float32).T.astype(np.float16)     # (D, HD)
    wv_c = wv_c.reshape(DT, P, HD).transpose(1, 0, 2).reshape(P, DT * HD)
    b1 = np.concatenate([np.asarray(bq, np.float32) * s,
                         np.asarray(bk, np.float32)]).reshape(P, 1)
    bv_f = np.asarray(bv, np.float32)
    bv_c = np.concatenate([bv_f, bv_f]).reshape(P, 1)
    # x (B,T,D) -> xT (B, P, NCH*DT*CH): xT[b,p,(ci,n,t)] = x[b, ci*CH+t, n*P+p]
    xT = np.swapaxes(x, 1, 2).astype(np.float16)               # (B, D, T)
    xT = xT.reshape(B, DT, P, NCH, CH)                         # D=(n,p), T=(ci,t)
    xT = xT.transpose(0, 2, 3, 1, 4).reshape(B, P, NCH * DT * CH)
    head = np.concatenate(
        [w1, wv_c, b1.astype(np.float16), bv_c.astype(np.float16)], axis=1)
    nx0 = DT * CH
    wb_b = np.ascontiguousarray(np.concatenate(
        [np.broadcast_to(head, (B,) + head.shape), xT[:, :, :nx0]], axis=2))
    xT_b = np.ascontiguousarray(xT[:, :, nx0:])

    nc = _build_module()
    in_maps = [{"xT": xT_b[b], "wb": wb_b[b]} for b in range(B)]
    res = None
    for attempt in range(3):
        try:
            res = run_bass_kernel_spmd(nc, in_maps, core_ids=list(range(B)))
            break
        except Exception:
            # transient device wedges (NRT_EXEC_UNIT_UNRECOVERABLE) happen;
            # rebuild the module and retry on a clean execution. A wedge can
            # also break the NTFF profile hook (rc=-1), so drop tracing for
            # the retries - correctness first.
            if attempt == 2:
                raise
            os.environ["BASS_NEVER_TRACE"] = "1"
            nc = _build_module()
    LAST_RESULTS = res

    out = np.empty((B, T, HD), dtype=np.float32)
    for b in range(B):
        oT = res.results[b]["outT"]  # (65, T): rows 0..63 = O^T, row 64 = denom
        out[b] = (oT[:HD] / oT[HD:HD + 1]).T
    return out


# revision 50
# speedup vs baseline: 1.1822x; 1.0229x over previous
"""Trainium2 Bass kernel for a single causal attention head.

Problem: x:(8,2048,1024) f32, per-head projections wq/wk/wv:(64,1024),
biases (64,). Output: softmax(causal(q k^T / sqrt(64))) @ v : (8,2048,64).

Strategy:
  - Data-parallel: batch b -> core b (8 cores, 1 batch each).
  - Host prep: x[b] transposed to xT:(1024,2048) fp16 (contraction dim D on
    SBUF partitions); Q/K weights shipped stacked as [wq|wk] (fp16, 1/sqrt(64)
    folded into wq).
  - Device (per core):
      * Input DMA split across both HWDGE rings: x quarter-chunks on the sync
        ring, weights/biases on the scalar ring, so xq0 lands ~5us earlier
        than a single serialized ring.
      * qk1 = [wq|wk]^T.T @ xT: rows 0-63 = Q^T, rows 64-127 = K^T (PSUM
        accumulate over 8 d-tiles, fp16 matmuls, N=512 chunks).
      * qk2 = half-swapped copy of qk1: K^T on rows 0-63, Q^T on rows 64-127.
        Both orders exist on both partition halves -> scores for TWO k-tiles
        run concurrently via PE row packing.
      * vT transposed back to (T,64) tiles via PE transpose, augmented with a
        ones column (softmax denominator rides along the PV matmul).
      * S^T[j,i] = sum_h K^T[h,j] Q^T[h,i] transposed-scores layout; P^T =
        exp(S^T) on ACT; causal mask = GPSIMD affine_select zeroing P^T in
        the 128x128 diagonal triangle blocks only (fully-masked column
        prefixes of diagonal tiles are skipped by S/exp/PV instead of
        computed-then-zeroed).
      * O^T_aug[65, T] accumulated in PSUM over k-tiles; row 64 = sum_j P^T.
        PSUM accumulate is per-element (has_written), so later k-tiles may
        touch columns the first matmul didn't.
      * causal skip: k-tiles entirely above the diagonal never computed;
        diagonal tiles compute only their live column suffix.
      * emission order interleaves chunk ci+1 projections ahead of chunk ci
        attention so the scalar engine (exp = the metronome) never starves.
  - Host post: out[b] = (O^T[0:64] / O^T[64:65]).T  (softmax normalization).
"""

import numpy as np

B, T, D, HD = 8, 2048, 1024, 64
P = 128          # SBUF partitions
CH = 512         # q-chunk (matmul moving dim)
NCH = T // CH    # 4
DT = D // P      # 8 d-tiles
NKT = T // P     # 16 k-tiles

LAST_RESULTS = None      # BassKernelResults of the most recent run (for test.py)


def _build_module(legalize=True):
    import concourse.bass as bass
    import concourse.mybir as mybir
    from concourse.tile import TileContext

    from concourse.masks import make_identity
    F32 = mybir.dt.float32
    F16 = mybir.dt.float16

    nc = bass.Bass("TRN2", target_bir_lowering=True)

    # All inputs host-pre-arranged to the exact SBUF layout so every DMA is
    # contiguous per partition (128 big descriptors, not 1024 tiny gathers —
    # the rearrange-in-DMA variant was descriptor-bound and took ~10us).
    # First transfer = [w1 | wv | b1 | bv | x-chunk0] per partition: ONE DMA
    # carrying exactly what the first projection needs, at full ring
    # bandwidth with a single completion receipt. Later x chunks follow
    # serially on the same ring in consumption order.
    WQOF = DT * P              # w1 cols
    BOF = WQOF + DT * HD       # b1 at BOF, bv at BOF+1 (wv starts at WQOF)
    XOF = BOF + 2              # chunk-0 x cols
    wb = nc.dram_tensor("wb", (P, XOF + DT * CH), F16, kind="ExternalInput")
    xT = nc.dram_tensor("xT", (P, (NCH - 1) * DT * CH), F16,
                        kind="ExternalInput")
    outT = nc.dram_tensor("outT", (HD + 1, T), F32, kind="ExternalOutput")

    with TileContext(nc) as tc:
        with (
            tc.tile_pool(name="const", bufs=1) as const,
            tc.tile_pool(name="acts", bufs=1) as acts,
            tc.tile_pool(name="proj_ps", bufs=1, space="PSUM") as proj_ps,
            tc.tile_pool(name="tr_ps", bufs=1, space="PSUM") as tr_ps,
            tc.tile_pool(name="s_ps", bufs=2, space="PSUM") as s_ps,
            tc.tile_pool(name="o_ps", bufs=2, space="PSUM") as o_ps,
            tc.tile_pool(name="pwork", bufs=20) as pwork,
            tc.tile_pool(name="owork", bufs=2) as owork,
        ):
            ident = const.tile([P, P], F32, name="ident")
            make_identity(nc, ident)

            # ---- weights/biases on the GPSIMD (SWDGE) ring so they drain in
            # parallel with the x quarters on the sync HWDGE ring. (The
            # scalar HWDGE ring is NOT used for input: a DIRECT2D occupies
            # the ACT queue for the whole transfer and would stall exps.)
            # Input DMA runs at the shared-HBM roofline (~310 GB/s/core with
            # all 8 cores loading at once) - 4.5MB takes ~14us no matter how
            # it's split. The blob rides the scalar ring with xq1 behind it
            # (that queue must drain before the first real exp ~17us); the
            # other chunks serialize on the sync ring in consumption order.
            wb_sb = const.tile([P, XOF + DT * CH], F16, name="wb_sb")
            nc.sync.dma_start(out=wb_sb[:], in_=wb[:, :])
            bias = const.tile([P, 2], F32, name="bias")
            nc.vector.tensor_copy(bias[:], wb_sb[:, BOF:BOF + 2])
            b1_sb = bias[:, 0:1]
            bv_sb = bias[:, 1:2]

            # xq[ci] = (tile, column offset of that chunk's x block)
            xq = [(wb_sb, XOF)]
            for ci in range(1, NCH):
                t = const.tile([P, DT * CH], F16, name=f"xq{ci}")
                nc.sync.dma_start(
                    out=t[:], in_=xT[:, (ci - 1) * DT * CH:ci * DT * CH])
                xq.append((t, 0))

            wscr = const.tile([P, 2 * P], F16, name="wscr")
            nc.vector.memset(wscr[:], 0.0)
            # dummy activation up front: walrus inserts the ~1.3us
            # ACT_TABLE_LOAD before the first exp; issuing it at t~0 hides
            # the load under the input DMA instead of the first real exp.
            edum = const.tile([P, 1], F16, name="edum")
            nc.scalar.activation(edum[:], wscr[:, 0:1],
                                 mybir.ActivationFunctionType.Exp)
            # HAM warm-up: short throwaway matmuls keep the PE active through
            # its 3.4us activity window so real matmuls run at the full
            # 2.4 GHz clock. From the bufs=2 pool so they pipeline instead of
            # serializing on slot release; sized to end about when xq0 lands.
            for wu in range(26):
                pswu = s_ps.tile([P, 2 * P], F32, name="warm", tag="sT")
                nc.tensor.matmul(pswu[:], wscr[:, 0:P], wscr[:],
                                 start=True, stop=True)

            # ---- activations ----
            qk1 = acts.tile([P, T], F16, name="qk1")
            qk2 = acts.tile([P, T], F16, name="qk2")
            # vT2: V^T for a chunk PAIR stacked on partition halves (rows
            # 0-63 = even chunk, 64-127 = odd chunk) -> one 128x128 PE
            # transpose yields TWO v_aug k-tiles.
            vT2 = acts.tile([P, T // 2], F32, name="vT2")
            v_aug = acts.tile([P, NKT, HD + 1], F16, name="v_aug")
            nc.vector.memset(v_aug[:, :, HD], 1.0)

            def qk_chunk(ci):
                cs = slice(ci * CH, (ci + 1) * CH)
                rhs, ro = xq[ci]
                ps = proj_ps.tile([P, CH], F32, name="proj", tag="proj")
                for d in range(DT):
                    nc.tensor.matmul(ps[:], wb_sb[:, d * P:(d + 1) * P],
                                     rhs[:, ro + d * CH:ro + (d + 1) * CH],
                                     start=(d == 0), stop=(d == DT - 1))
                nc.vector.tensor_scalar_add(qk1[:, cs], ps[:], b1_sb)
                # half-swapped copy: qk2 = [K^T; Q^T]. 64-partition DVE ops
                # read any aligned src half and write either dest half.
                nc.vector.tensor_copy(qk2[0:HD, cs], qk1[HD:P, cs])
                nc.vector.tensor_copy(qk2[HD:P, cs], qk1[0:HD, cs])

            def v_pair(ca, cb):
                # V projections for two chunks col-packed: chunk ca on array
                # columns 0-63, chunk cb on columns 64-127 -> the matmul pairs
                # overlap in the PE array; outputs land in disjoint halves of
                # one PSUM bank. The stacked vT2 halves then transpose two
                # k-tiles per 128x128 PE transpose.
                off = (ca // 2) * CH
                psv = proj_ps.tile([P, CH], F32, name="projv", tag="proj")
                (ta, oa), (tb, ob) = xq[ca], xq[cb]
                for d in range(DT):
                    wvd = wb_sb[:, WQOF + d * HD:WQOF + (d + 1) * HD]
                    nc.tensor.matmul(psv[0:HD, :], wvd,
                                     ta[:, oa + d * CH:oa + (d + 1) * CH],
                                     start=(d == 0), stop=(d == DT - 1))
                    nc.tensor.matmul(psv[HD:P, :], wvd,
                                     tb[:, ob + d * CH:ob + (d + 1) * CH],
                                     start=(d == 0), stop=(d == DT - 1))
                nc.vector.tensor_scalar_add(
                    vT2[0:HD, off:off + CH], psv[0:HD, :], bv_sb[0:HD])
                nc.vector.tensor_scalar_add(
                    vT2[HD:P, off:off + CH], psv[HD:P, :], bv_sb[HD:P])
                for i in range(4):
                    tp = tr_ps.tile([P, 2, HD], F32, name="vtr", tag="vtr")
                    nc.tensor.transpose(tp[:, :, :],
                                        vT2[:, off + i * P:off + (i + 1) * P],
                                        ident[:, :])
                    nc.vector.tensor_copy(
                        v_aug[:, 4 * ca + i:4 * cb + i + 1:4, 0:HD], tp[:, :, :])

            def _tri_mask(pt, col):
                # zero the strictly-above-diagonal half of a 128x128 block
                # whose query offset equals its key offset (iota = c - j).
                nc.gpsimd.affine_select(
                    out=pt[:, col:col + P], in_=pt[:, col:col + P],
                    compare_op=mybir.AluOpType.is_ge, fill=0.0,
                    base=0, pattern=[[1, P]], channel_multiplier=-1,
                )

            # per-chunk pt tiles handed from the S/exp units to the PV units
            pts = {}
            opst = {}

            def s_off(ci, j):
                # off-diagonal pair j: full 512-wide tiles, no mask
                cs = slice(ci * CH, (ci + 1) * CH)
                ka, kb = 2 * j, 2 * j + 1
                s2 = s_ps.tile([P, 2 * CH], F32, name="sT", tag="sT")
                nc.tensor.matmul(s2[:, 0:CH], qk2[0:HD, ka * P:(ka + 1) * P],
                                 qk1[0:HD, cs], start=True, stop=True)
                nc.tensor.matmul(s2[:, CH:2 * CH], qk1[HD:P, kb * P:(kb + 1) * P],
                                 qk2[HD:P, cs], start=True, stop=True)
                pt = pwork.tile([P, 2 * CH], F16, name="pT", tag="pT")
                nc.scalar.activation(pt[:], s2[:],
                                     mybir.ActivationFunctionType.Exp)
                pts.setdefault(ci, []).append(pt)

            def s_diagA(ci):
                # diagonal pair A: d0 full at cols [0:512]; d1's live columns
                # (queries 128..511) at cols [512:896]. One exp over [0:896].
                cs = slice(ci * CH, (ci + 1) * CH)
                k0 = 4 * ci
                s2 = s_ps.tile([P, 2 * CH], F32, name="sT", tag="sT")
                nc.tensor.matmul(s2[:, 0:CH], qk2[0:HD, k0 * P:(k0 + 1) * P],
                                 qk1[0:HD, cs], start=True, stop=True)
                nc.tensor.matmul(s2[:, CH:CH + 384],
                                 qk1[HD:P, (k0 + 1) * P:(k0 + 2) * P],
                                 qk2[HD:P, ci * CH + P:(ci + 1) * CH],
                                 start=True, stop=True)
                pt = pwork.tile([P, 2 * CH], F16, name="pT", tag="pT")
                nc.scalar.activation(pt[:, 0:CH + 384], s2[:, 0:CH + 384],
                                     mybir.ActivationFunctionType.Exp)
                _tri_mask(pt, 0)        # d0 triangle (queries 0..127)
                _tri_mask(pt, CH)       # d1 triangle (queries 128..255)
                pts.setdefault(ci, []).append(pt)

            def s_diagB(ci):
                # diagonal pair B: d2's live columns (queries 256..511) in
                # place at [256:512]; d3's (queries 384..511) at [512:640].
                # One exp over the contiguous live span [256:640].
                k0 = 4 * ci
                s2 = s_ps.tile([P, 2 * CH], F32, name="sT", tag="sT")
                nc.tensor.matmul(s2[:, 2 * P:CH],
                                 qk2[0:HD, (k0 + 2) * P:(k0 + 3) * P],
                                 qk1[0:HD, ci * CH + 2 * P:(ci + 1) * CH],
                                 start=True, stop=True)
                nc.tensor.matmul(s2[:, CH:CH + P],
                                 qk1[HD:P, (k0 + 3) * P:(k0 + 4) * P],
                                 qk2[HD:P, ci * CH + 3 * P:(ci + 1) * CH],
                                 start=True, stop=True)
                pt = pwork.tile([P, 2 * CH], F16, name="pT", tag="pT")
                nc.scalar.activation(pt[:, 2 * P:CH + P], s2[:, 2 * P:CH + P],
                                     mybir.ActivationFunctionType.Exp)
                _tri_mask(pt, 2 * P)    # d2 triangle (queries 256..383)
                _tri_mask(pt, CH)       # d3 triangle (queries 384..511)
                pts.setdefault(ci, []).append(pt)

            def pvu(ci, j):
                # PV consumption of pair j of chunk ci. Unit 2ci = diag A,
                # 2ci+1 = diag B; accumulation group opens at unit 0 and
                # closes on diag B's last matmul.
                if j == 0:
                    opst[ci] = o_ps.tile([HD + 1, CH], F32, name="oacc",
                                         tag="oacc")
                ops = opst[ci]
                pt = pts[ci][j]
                k0 = 4 * ci
                if j < 2 * ci:
                    ka, kb = 2 * j, 2 * j + 1
                    nc.tensor.matmul(ops[:], v_aug[:, ka, :], pt[:, 0:CH],
                                     start=(j == 0), stop=False)
                    nc.tensor.matmul(ops[:], v_aug[:, kb, :], pt[:, CH:2 * CH],
                                     start=False, stop=False)
                elif j == 2 * ci:
                    # d0 covers all 512 columns; later tiles touch only their
                    # live suffix (per-element has_written handles first-write)
                    nc.tensor.matmul(ops[:], v_aug[:, k0, :], pt[:, 0:CH],
                                     start=(j == 0), stop=False)
                    nc.tensor.matmul(ops[:, P:CH], v_aug[:, k0 + 1, :],
                                     pt[:, CH:CH + 384], start=False, stop=False)
                else:
                    nc.tensor.matmul(ops[:, 2 * P:CH], v_aug[:, k0 + 2, :],
                                     pt[:, 2 * P:CH], start=False, stop=False)
                    nc.tensor.matmul(ops[:, 3 * P:CH], v_aug[:, k0 + 3, :],
                                     pt[:, CH:CH + P], start=False, stop=True)

            def osb_u(ci):
                cs = slice(ci * CH, (ci + 1) * CH)
                osb = owork.tile([HD + 1, CH], F32, name="osb", tag="osb")
                nc.vector.tensor_copy(osb[:], opst.pop(ci)[:])
                nc.sync.dma_start(out=outT[:, cs], in_=osb[:])

            # Emission order = scheduler priority, in bands: QK projections
            # and S/exp/mask (the ACT metronome's supply line) first, then
            # PVs (drain pt slots), then V projections+transposes last —
            # they have slack until their pv() and fill PE gaps, gated only
            # by data deps.
            qk_chunk(0)
            s_diagA(0)
            s_diagB(0)
            qk_chunk(1)
            s_off(1, 0)
            s_off(1, 1)
            s_diagA(1)
            s_diagB(1)
            v_pair(0, 1)
            qk_chunk(2)
            # PV units ride BETWEEN S pairs: the engine queues are strict
            # FIFO at runtime, and an S pair at the queue head waiting for
            # its PSUM slot (drip-fed by the exp stream) would trap every
            # later-emitted instruction behind it. Interleaving keeps ready
            # PV work ahead of each slot-waiting S pair.
            s_off(2, 0)
            pvu(0, 0)
            s_off(2, 1)
            pvu(0, 1)
            osb_u(0)
            s_off(2, 2)
            s_off(2, 3)
            s_diagA(2)
            s_diagB(2)
            qk_chunk(3)
            v_pair(2, 3)
            s_off(3, 0)
            pvu(1, 0)
            s_off(3, 1)
            pvu(1, 1)
            s_off(3, 2)
            pvu(1, 2)
            s_off(3, 3)
            pvu(1, 3)
            osb_u(1)
            s_off(3, 4)
            pvu(2, 0)
            s_off(3, 5)
            pvu(2, 1)
            s_diagA(3)
            pvu(2, 2)
            s_diagB(3)
            pvu(2, 3)
            pvu(2, 4)
            pvu(2, 5)
            osb_u(2)
            for j in range(8):
                pvu(3, j)
            osb_u(3)

    if legalize:
        _legalize_waits(nc, mybir)
    return nc


def _legalize_waits(nc, mybir):
    """Split multi-wait instructions for the XLA-route walrus codegen.

    The TPB EVENTS struct holds one semaphore wait per instruction and this
    pipeline's codegen refuses >1. Hoist extra waits onto standalone
    EventSemaphore instructions on the same engine queue right before the
    instruction - semantically identical, the queue stalls there.
    """
    n = 0
    for f in nc.m.functions:
        for b in f.blocks:
            out = []
            changed = False
            for inst in b.instructions:
                si = inst.sync_info
                waits = list(si.on_wait) if si is not None and si.on_wait else []
                if len(waits) > 1:
                    changed = True
                    for w in waits[:-1]:
                        n += 1
                        out.append(mybir.InstEventSemaphore(
                            name=f"waitfix{n}_{inst.name}",
                            engine=inst.engine,
                            sync_info=mybir.SyncInfo(on_wait=[w], on_update=[]),
                        ))
                    inst.sync_info = mybir.SyncInfo(
                        on_wait=waits[-1:],
                        on_update=list(si.on_update or []),
                    )
                out.append(inst)
            if changed:
                b.instructions = out
    return n


def kernel(x, wq, bq, wk, bk, wv, bv):
    global LAST_RESULTS
    import os
    os.environ.setdefault("JAX_PLATFORMS", "")
    from concourse.bass_utils import run_bass_kernel_spmd

    x = np.asarray(x, dtype=np.float32)
    s = np.float32(1.0 / np.sqrt(HD))
    wq_s = np.asarray(wq, np.float32) * s
    wk_f = np.asarray(wk, np.float32)
    # (D, 128) -> (128, DT*128): partition p holds rows {n*128+p}
    w1 = np.concatenate([wq_s, wk_f], 0).T.astype(np.float16)  # (D, P)
    w1 = w1.reshape(DT, P, P).transpose(1, 0, 2).reshape(P, DT * P)
    wv_c = np.asarray(wv, np.float32).T.astype(np.float16)     # (D, HD)
    wv_c = wv_c.reshape(DT, P, HD).transpose(1, 0, 2).reshape(P, DT * HD)
    b1 = np.concatenate([np.asarray(bq, np.float32) * s,
                         np.asarray(bk, np.float32)]).reshape(P, 1)
    bv_f = np.asarray(bv, np.float32)
    bv_c = np.concatenate([bv_f, bv_f]).reshape(P, 1)
    # x (B,T,D) -> xT (B, P, NCH*DT*CH): xT[b,p,(ci,n,t)] = x[b, ci*CH+t, n*P+p]
    xT = np.swapaxes(x, 1, 2).astype(np.float16)               # (B, D, T)
    xT = xT.reshape(B, DT, P, NCH, CH)                         # D=(n,p), T=(ci,t)
    xT = xT.transpose(0, 2, 3, 1, 4).reshape(B, P, NCH * DT * CH)
    head = np.concatenate(
        [w1, wv_c, b1.astype(np.float16), bv_c.astype(np.float16)], axis=1)
    nx0 = DT * CH
    wb_b = np.ascontiguousarray(np.concatenate(
        [np.broadcast_to(head, (B,) + head.shape), xT[:, :, :nx0]], axis=2))
    xT_b = np.ascontiguousarray(xT[:, :, nx0:])

    nc = _build_module()
    in_maps = [{"xT": xT_b[b], "wb": wb_b[b]} for b in range(B)]
    res = None
    for attempt in range(3):
        try:
            res = run_bass_kernel_spmd(nc, in_maps, core_ids=list(range(B)))
            break
        except Exception:
            # transient device wedges (NRT_EXEC_UNIT_UNRECOVERABLE) happen;
            # rebuild the module and retry on a clean execution. A wedge can
            # also break the NTFF profile hook (rc=-1), so drop tracing for
            # the retries - correctness first.
            if attempt == 2:
                raise
            os.environ["BASS_NEVER_TRACE"] = "1"
            nc = _build_module()
    LAST_RESULTS = res

    out = np.empty((B, T, HD), dtype=np.float32)
    for b in range(B):
        oT = res.results[b]["outT"]  # (65, T): rows 0..63 = O^T, row 64 = denom
        out[b] = (oT[:HD] / oT[HD:HD + 1]).T
    return out


# revision 52
# speedup vs baseline: 1.2198x; 1.0318x over previous
"""Trainium2 Bass kernel for a single causal attention head.

Problem: x:(8,2048,1024) f32, per-head projections wq/wk/wv:(64,1024),
biases (64,). Output: softmax(causal(q k^T / sqrt(64))) @ v : (8,2048,64).

Strategy:
  - Data-parallel: batch b -> core b (8 cores, 1 batch each).
  - Host prep: x[b] transposed to xT:(1024,2048) fp16 (contraction dim D on
    SBUF partitions); Q/K weights shipped stacked as [wq|wk] (fp16, 1/sqrt(64)
    folded into wq).
  - Device (per core):
      * Input DMA split across both HWDGE rings: x quarter-chunks on the sync
        ring, weights/biases on the scalar ring, so xq0 lands ~5us earlier
        than a single serialized ring.
      * qk1 = [wq|wk]^T.T @ xT: rows 0-63 = Q^T, rows 64-127 = K^T (PSUM
        accumulate over 8 d-tiles, fp16 matmuls, N=512 chunks).
      * qk2 = half-swapped copy of qk1: K^T on rows 0-63, Q^T on rows 64-127.
        Both orders exist on both partition halves -> scores for TWO k-tiles
        run concurrently via PE row packing.
      * vT transposed back to (T,64) tiles via PE transpose, augmented with a
        ones column (softmax denominator rides along the PV matmul).
      * S^T[j,i] = sum_h K^T[h,j] Q^T[h,i] transposed-scores layout; P^T =
        exp(S^T) on ACT; causal mask = GPSIMD affine_select zeroing P^T in
        the 128x128 diagonal triangle blocks only (fully-masked column
        prefixes of diagonal tiles are skipped by S/exp/PV instead of
        computed-then-zeroed).
      * O^T_aug[65, T] accumulated in PSUM over k-tiles; row 64 = sum_j P^T.
        PSUM accumulate is per-element (has_written), so later k-tiles may
        touch columns the first matmul didn't.
      * causal skip: k-tiles entirely above the diagonal never computed;
        diagonal tiles compute only their live column suffix.
      * emission order interleaves chunk ci+1 projections ahead of chunk ci
        attention so the scalar engine (exp = the metronome) never starves.
  - Host post: out[b] = (O^T[0:64] / O^T[64:65]).T  (softmax normalization).
"""

import numpy as np

B, T, D, HD = 8, 2048, 1024, 64
P = 128          # SBUF partitions
CH = 512         # q-chunk (matmul moving dim)
NCH = T // CH    # 4
DT = D // P      # 8 d-tiles
NKT = T // P     # 16 k-tiles

LAST_RESULTS = None      # BassKernelResults of the most recent run (for test.py)


def _build_module(legalize=True):
    import concourse.bass as bass
    import concourse.mybir as mybir
    from concourse.tile import TileContext

    from concourse.masks import make_identity
    F32 = mybir.dt.float32
    F16 = mybir.dt.float16

    nc = bass.Bass("TRN2", target_bir_lowering=True)

    # All inputs host-pre-arranged to the exact SBUF layout so every DMA is
    # contiguous per partition (128 big descriptors, not 1024 tiny gathers —
    # the rearrange-in-DMA variant was descriptor-bound and took ~10us).
    # First transfer = [w1 | wv | b1 | bv | x-chunk0] per partition: ONE DMA
    # carrying exactly what the first projection needs, at full ring
    # bandwidth with a single completion receipt. Later x chunks follow
    # serially on the same ring in consumption order.
    WQOF = DT * P              # w1 cols
    BOF = WQOF + DT * HD       # b1 at BOF, bv at BOF+1 (wv starts at WQOF)
    XOF = BOF + 2              # chunk-0 x cols
    wb = nc.dram_tensor("wb", (P, XOF + DT * CH), F16, kind="ExternalInput")
    xT = nc.dram_tensor("xT", (P, (NCH - 1) * DT * CH), F16,
                        kind="ExternalInput")
    outT = nc.dram_tensor("outT", (HD + 1, T), F32, kind="ExternalOutput")

    with TileContext(nc) as tc:
        with (
            tc.tile_pool(name="const", bufs=1) as const,
            tc.tile_pool(name="acts", bufs=1) as acts,
            tc.tile_pool(name="proj_ps", bufs=1, space="PSUM") as proj_ps,
            tc.tile_pool(name="tr_ps", bufs=1, space="PSUM") as tr_ps,
            tc.tile_pool(name="s_ps", bufs=2, space="PSUM") as s_ps,
            tc.tile_pool(name="o_ps", bufs=2, space="PSUM") as o_ps,
            tc.tile_pool(name="pwork", bufs=20) as pwork,
            tc.tile_pool(name="owork", bufs=2) as owork,
        ):
            ident = const.tile([P, P], F32, name="ident")
            make_identity(nc, ident)

            # ---- weights/biases on the GPSIMD (SWDGE) ring so they drain in
            # parallel with the x quarters on the sync HWDGE ring. (The
            # scalar HWDGE ring is NOT used for input: a DIRECT2D occupies
            # the ACT queue for the whole transfer and would stall exps.)
            # Input DMA runs at the shared-HBM roofline (~310 GB/s/core with
            # all 8 cores loading at once) - 4.5MB takes ~14us no matter how
            # it's split. The blob rides the scalar ring with xq1 behind it
            # (that queue must drain before the first real exp ~17us); the
            # other chunks serialize on the sync ring in consumption order.
            wb_sb = const.tile([P, XOF + DT * CH], F16, name="wb_sb")
            nc.sync.dma_start(out=wb_sb[:], in_=wb[:, :])
            bias = const.tile([P, 2], F32, name="bias")
            nc.vector.tensor_copy(bias[:], wb_sb[:, BOF:BOF + 2])
            b1_sb = bias[:, 0:1]
            bv_sb = bias[:, 1:2]

            # xq[ci] = (tile, column offset of that chunk's x block)
            xq = [(wb_sb, XOF)]
            for ci in range(1, NCH):
                t = const.tile([P, DT * CH], F16, name=f"xq{ci}")
                nc.sync.dma_start(
                    out=t[:], in_=xT[:, (ci - 1) * DT * CH:ci * DT * CH])
                xq.append((t, 0))

            wscr = const.tile([P, 2 * P], F16, name="wscr")
            nc.vector.memset(wscr[:], 0.0)
            # dummy activation up front: walrus inserts the ~1.3us
            # ACT_TABLE_LOAD before the first exp; issuing it at t~0 hides
            # the load under the input DMA instead of the first real exp.
            edum = const.tile([P, 1], F16, name="edum")
            nc.scalar.activation(edum[:], wscr[:, 0:1],
                                 mybir.ActivationFunctionType.Exp)
            # HAM warm-up: short throwaway matmuls keep the PE active through
            # its 3.4us activity window so real matmuls run at the full
            # 2.4 GHz clock. From the bufs=2 pool so they pipeline instead of
            # serializing on slot release; sized to end about when xq0 lands.
            for wu in range(26):
                pswu = s_ps.tile([P, 2 * P], F32, name="warm", tag="sT")
                nc.tensor.matmul(pswu[:], wscr[:, 0:P], wscr[:],
                                 start=True, stop=True)

            # ---- activations ----
            qk1 = acts.tile([P, T], F16, name="qk1")
            qk2 = acts.tile([P, T], F16, name="qk2")
            # vT2: V^T for a chunk PAIR stacked on partition halves (rows
            # 0-63 = even chunk, 64-127 = odd chunk) -> one 128x128 PE
            # transpose yields TWO v_aug k-tiles.
            vT2 = acts.tile([P, T // 2], F32, name="vT2")
            v_aug = acts.tile([P, NKT, HD + 1], F16, name="v_aug")
            nc.vector.memset(v_aug[:, :, HD], 1.0)

            def qk_chunk(ci):
                cs = slice(ci * CH, (ci + 1) * CH)
                rhs, ro = xq[ci]
                ps = proj_ps.tile([P, CH], F32, name="proj", tag="proj")
                for d in range(DT):
                    nc.tensor.matmul(ps[:], wb_sb[:, d * P:(d + 1) * P],
                                     rhs[:, ro + d * CH:ro + (d + 1) * CH],
                                     start=(d == 0), stop=(d == DT - 1))
                nc.vector.tensor_scalar_add(qk1[:, cs], ps[:], b1_sb)
                # half-swapped copy: qk2 = [K^T; Q^T]. 64-partition DVE ops
                # read any aligned src half and write either dest half.
                nc.vector.tensor_copy(qk2[0:HD, cs], qk1[HD:P, cs])
                nc.vector.tensor_copy(qk2[HD:P, cs], qk1[0:HD, cs])

            vps = {}

            def v_mms(ca, cb):
                # V projections for two chunks col-packed: chunk ca on array
                # columns 0-63, chunk cb on columns 64-127 -> the matmul pairs
                # overlap in the PE array; outputs land in disjoint halves of
                # one PSUM bank.
                psv = proj_ps.tile([P, CH], F32, name="projv", tag="proj")
                vps[ca] = psv
                (ta, oa), (tb, ob) = xq[ca], xq[cb]
                for d in range(DT):
                    wvd = wb_sb[:, WQOF + d * HD:WQOF + (d + 1) * HD]
                    nc.tensor.matmul(psv[0:HD, :], wvd,
                                     ta[:, oa + d * CH:oa + (d + 1) * CH],
                                     start=(d == 0), stop=(d == DT - 1))
                    nc.tensor.matmul(psv[HD:P, :], wvd,
                                     tb[:, ob + d * CH:ob + (d + 1) * CH],
                                     start=(d == 0), stop=(d == DT - 1))

            def v_fin(ca, cb):
                # bias add into the stacked vT2 halves, then one 128x128 PE
                # transpose per k-tile PAIR (two v_aug tiles per transpose).
                off = (ca // 2) * CH
                psv = vps.pop(ca)
                nc.vector.tensor_scalar_add(
                    vT2[0:HD, off:off + CH], psv[0:HD, :], bv_sb[0:HD])
                nc.vector.tensor_scalar_add(
                    vT2[HD:P, off:off + CH], psv[HD:P, :], bv_sb[HD:P])
                for i in range(4):
                    tp = tr_ps.tile([P, 2, HD], F32, name="vtr", tag="vtr")
                    nc.tensor.transpose(tp[:, :, :],
                                        vT2[:, off + i * P:off + (i + 1) * P],
                                        ident[:, :])
                    nc.vector.tensor_copy(
                        v_aug[:, 4 * ca + i:4 * cb + i + 1:4, 0:HD], tp[:, :, :])

            def _tri_mask(pt, col):
                # zero the strictly-above-diagonal half of a 128x128 block
                # whose query offset equals its key offset (iota = c - j).
                nc.gpsimd.affine_select(
                    out=pt[:, col:col + P], in_=pt[:, col:col + P],
                    compare_op=mybir.AluOpType.is_ge, fill=0.0,
                    base=0, pattern=[[1, P]], channel_multiplier=-1,
                )

            # per-chunk pt tiles handed from the S/exp units to the PV units
            pts = {}
            opst = {}

            def s_off(ci, j):
                # off-diagonal pair j: full 512-wide tiles, no mask
                cs = slice(ci * CH, (ci + 1) * CH)
                ka, kb = 2 * j, 2 * j + 1
                s2 = s_ps.tile([P, 2 * CH], F32, name="sT", tag="sT")
                nc.tensor.matmul(s2[:, 0:CH], qk2[0:HD, ka * P:(ka + 1) * P],
                                 qk1[0:HD, cs], start=True, stop=True)
                nc.tensor.matmul(s2[:, CH:2 * CH], qk1[HD:P, kb * P:(kb + 1) * P],
                                 qk2[HD:P, cs], start=True, stop=True)
                pt = pwork.tile([P, 2 * CH], F16, name="pT", tag="pT")
                nc.scalar.activation(pt[:], s2[:],
                                     mybir.ActivationFunctionType.Exp)
                pts.setdefault(ci, []).append(pt)

            def s_diagA(ci):
                # diagonal pair A: d0 full at cols [0:512]; d1's live columns
                # (queries 128..511) at cols [512:896]. One exp over [0:896].
                cs = slice(ci * CH, (ci + 1) * CH)
                k0 = 4 * ci
                s2 = s_ps.tile([P, 2 * CH], F32, name="sT", tag="sT")
                nc.tensor.matmul(s2[:, 0:CH], qk2[0:HD, k0 * P:(k0 + 1) * P],
                                 qk1[0:HD, cs], start=True, stop=True)
                nc.tensor.matmul(s2[:, CH:CH + 384],
                                 qk1[HD:P, (k0 + 1) * P:(k0 + 2) * P],
                                 qk2[HD:P, ci * CH + P:(ci + 1) * CH],
                                 start=True, stop=True)
                pt = pwork.tile([P, 2 * CH], F16, name="pT", tag="pT")
                nc.scalar.activation(pt[:, 0:CH + 384], s2[:, 0:CH + 384],
                                     mybir.ActivationFunctionType.Exp)
                _tri_mask(pt, 0)        # d0 triangle (queries 0..127)
                _tri_mask(pt, CH)       # d1 triangle (queries 128..255)
                pts.setdefault(ci, []).append(pt)

            def s_diagB(ci):
                # diagonal pair B: d2's live columns (queries 256..511) in
                # place at [256:512]; d3's (queries 384..511) at [512:640].
                # One exp over the contiguous live span [256:640].
                k0 = 4 * ci
                s2 = s_ps.tile([P, 2 * CH], F32, name="sT", tag="sT")
                nc.tensor.matmul(s2[:, 2 * P:CH],
                                 qk2[0:HD, (k0 + 2) * P:(k0 + 3) * P],
                                 qk1[0:HD, ci * CH + 2 * P:(ci + 1) * CH],
                                 start=True, stop=True)
                nc.tensor.matmul(s2[:, CH:CH + P],
                                 qk1[HD:P, (k0 + 3) * P:(k0 + 4) * P],
                                 qk2[HD:P, ci * CH + 3 * P:(ci + 1) * CH],
                                 start=True, stop=True)
                pt = pwork.tile([P, 2 * CH], F16, name="pT", tag="pT")
                nc.scalar.activation(pt[:, 2 * P:CH + P], s2[:, 2 * P:CH + P],
                                     mybir.ActivationFunctionType.Exp)
                _tri_mask(pt, 2 * P)    # d2 triangle (queries 256..383)
                _tri_mask(pt, CH)       # d3 triangle (queries 384..511)
                pts.setdefault(ci, []).append(pt)

            def pvu(ci, j):
                # PV consumption of pair j of chunk ci. Unit 2ci = diag A,
                # 2ci+1 = diag B; accumulation group opens at unit 0 and
                # closes on diag B's last matmul.
                if j == 0:
                    opst[ci] = o_ps.tile([HD + 1, CH], F32, name="oacc",
                                         tag="oacc")
                ops = opst[ci]
                pt = pts[ci][j]
                k0 = 4 * ci
                if j < 2 * ci:
                    ka, kb = 2 * j, 2 * j + 1
                    nc.tensor.matmul(ops[:], v_aug[:, ka, :], pt[:, 0:CH],
                                     start=(j == 0), stop=False)
                    nc.tensor.matmul(ops[:], v_aug[:, kb, :], pt[:, CH:2 * CH],
                                     start=False, stop=False)
                elif j == 2 * ci:
                    # d0 covers all 512 columns; later tiles touch only their
                    # live suffix (per-element has_written handles first-write)
                    nc.tensor.matmul(ops[:], v_aug[:, k0, :], pt[:, 0:CH],
                                     start=(j == 0), stop=False)
                    nc.tensor.matmul(ops[:, P:CH], v_aug[:, k0 + 1, :],
                                     pt[:, CH:CH + 384], start=False, stop=False)
                else:
                    nc.tensor.matmul(ops[:, 2 * P:CH], v_aug[:, k0 + 2, :],
                                     pt[:, 2 * P:CH], start=False, stop=False)
                    nc.tensor.matmul(ops[:, 3 * P:CH], v_aug[:, k0 + 3, :],
                                     pt[:, CH:CH + P], start=False, stop=True)

            def osb_u(ci):
                cs = slice(ci * CH, (ci + 1) * CH)
                osb = owork.tile([HD + 1, CH], F32, name="osb", tag="osb")
                nc.vector.tensor_copy(osb[:], opst.pop(ci)[:])
                nc.sync.dma_start(out=outT[:, cs], in_=osb[:])

            # Emission order = scheduler priority, in bands: QK projections
            # and S/exp/mask (the ACT metronome's supply line) first, then
            # PVs (drain pt slots), then V projections+transposes last —
            # they have slack until their pv() and fill PE gaps, gated only
            # by data deps.
            # Emission order = FIFO order per engine at runtime (the Tile
            # scheduler's cost model doesn't know real DMA latencies), so:
            # projections and S pairs lead each chunk; the V-projection and
            # PV units slot into the gaps where S pairs wait for their PSUM
            # slots (drip-fed by the exp stream); nothing multi-us and
            # low-priority may sit ahead of chunk-critical work.
            qk_chunk(0)
            s_diagA(0)
            s_diagB(0)
            qk_chunk(1)
            s_off(1, 0)
            s_off(1, 1)
            qk_chunk(2)
            s_diagA(1)
            s_diagB(1)
            s_off(2, 0)
            s_off(2, 1)
            v_mms(0, 1)
            v_fin(0, 1)
            s_off(2, 2)
            pvu(0, 0)
            s_off(2, 3)
            pvu(0, 1)
            osb_u(0)
            s_diagA(2)
            s_diagB(2)
            qk_chunk(3)
            v_mms(2, 3)
            v_fin(2, 3)
            s_off(3, 0)
            pvu(1, 0)
            s_off(3, 1)
            pvu(1, 1)
            s_off(3, 2)
            pvu(1, 2)
            s_off(3, 3)
            pvu(1, 3)
            osb_u(1)
            s_off(3, 4)
            pvu(2, 0)
            s_off(3, 5)
            pvu(2, 1)
            s_diagA(3)
            pvu(2, 2)
            s_diagB(3)
            pvu(2, 3)
            pvu(2, 4)
            pvu(2, 5)
            osb_u(2)
            for j in range(8):
                pvu(3, j)
            osb_u(3)

    if legalize:
        _legalize_waits(nc, mybir)
    return nc


def _legalize_waits(nc, mybir):
    """Split multi-wait instructions for the XLA-route walrus codegen.

    The TPB EVENTS struct holds one semaphore wait per instruction and this
    pipeline's codegen refuses >1. Hoist extra waits onto standalone
    EventSemaphore instructions on the same engine queue right before the
    instruction - semantically identical, the queue stalls there.
    """
    n = 0
    for f in nc.m.functions:
        for b in f.blocks:
            out = []
            changed = False
            for inst in b.instructions:
                si = inst.sync_info
                waits = list(si.on_wait) if si is not None and si.on_wait else []
                if len(waits) > 1:
                    changed = True
                    for w in waits[:-1]:
                        n += 1
                        out.append(mybir.InstEventSemaphore(
                            name=f"waitfix{n}_{inst.name}",
                            engine=inst.engine,
                            sync_info=mybir.SyncInfo(on_wait=[w], on_update=[]),
                        ))
                    inst.sync_info = mybir.SyncInfo(
                        on_wait=waits[-1:],
                        on_update=list(si.on_update or []),
                    )
                out.append(inst)
            if changed:
                b.instructions = out
    return n


def kernel(x, wq, bq, wk, bk, wv, bv):
    global LAST_RESULTS
    import os
    os.environ.setdefault("JAX_PLATFORMS", "")
    from concourse.bass_utils import run_bass_kernel_spmd

    x = np.asarray(x, dtype=np.float32)
    s = np.float32(1.0 / np.sqrt(HD))
    wq_s = np.asarray(wq, np.float32) * s
    wk_f = np.asarray(wk, np.float32)
    # (D, 128) -> (128, DT*128): partition p holds rows {n*128+p}
    w1 = np.concatenate([wq_s, wk_f], 0).T.astype(np.float16)  # (D, P)
    w1 = w1.reshape(DT, P, P).transpose(1, 0, 2).reshape(P, DT * P)
    wv_c = np.asarray(wv, np.float32).T.astype(np.float16)     # (D, HD)
    wv_c = wv_c.reshape(DT, P, HD).transpose(1, 0, 2).reshape(P, DT * HD)
    b1 = np.concatenate([np.asarray(bq, np.float32) * s,
                         np.asarray(bk, np.float32)]).reshape(P, 1)
    bv_f = np.asarray(bv, np.float32)
    bv_c = np.concatenate([bv_f, bv_f]).reshape(P, 1)
    # x (B,T,D) -> xT (B, P, NCH*DT*CH): xT[b,p,(ci,n,t)] = x[b, ci*CH+t, n*P+p]
    xT = np.swapaxes(x, 1, 2).astype(np.float16)               # (B, D, T)
    xT = xT.reshape(B, DT, P, NCH, CH)                         # D=(n,p), T=(ci,t)
    xT = xT.transpose(0, 2, 3, 1, 4).reshape(B, P, NCH * DT * CH)
    head = np.concatenate(
        [w1, wv_c, b1.astype(np.float16), bv_c.astype(np.float16)], axis=1)
    nx0 = DT * CH
    wb_b = np.ascontiguousarray(np.concatenate(
        [np.broadcast_to(head, (B,) + head.shape), xT[:, :, :nx0]], axis=2))
    xT_b = np.ascontiguousarray(xT[:, :, nx0:])

    nc = _build_module()
    in_maps = [{"xT": xT_b[b], "wb": wb_b[b]} for b in range(B)]
    res = None
    for attempt in range(3):
        try:
            res = run_bass_kernel_spmd(nc, in_maps, core_ids=list(range(B)))
            break
        except Exception:
            # transient device wedges (NRT_EXEC_UNIT_UNRECOVERABLE) happen;
            # rebuild the module and retry on a clean execution. A wedge can
            # also break the NTFF profile hook (rc=-1), so drop tracing for
            # the retries - correctness first.
            if attempt == 2:
                raise
            os.environ["BASS_NEVER_TRACE"] = "1"
            nc = _build_module()
    LAST_RESULTS = res

    out = np.empty((B, T, HD), dtype=np.float32)
    for b in range(B):
        oT = res.results[b]["outT"]  # (65, T): rows 0..63 = O^T, row 64 = denom
        out[b] = (oT[:HD] / oT[HD:HD + 1]).T
    return out
